# revision 1
# baseline (speedup 1.0000x reference)
"""Trainium2 Bass kernel for nn_MoEALU (soft ripple-carry byte adder).

Math (equivalent to reference, exploiting table structure):
  - b2n contraction == segmented sums of the 256-byte distribution
    (high nibble: 16 contiguous groups; low nibble: stride-16 groups).
  - add_table/carry_table contraction of w = x (x) y (x) cin decomposes via the
    linear convolution z = conv(x, y) (z[31]==0 pad):
        u[m]  = z[m] + z[m+16]            (mod-16 fold)
        s     = u*c0 + rot1(u)*c1         (soft sum logits)
        cr0   = Z0*c0 + (Z0 - z15)*c1     (Z0 = sum z[0:16])
        cr1   = Z1*c0 + (Z1 + z15)*c1     (Z1 = sum z[16:32])
  - n2b contraction == broadcast add: o[i,j] = sh[i] + sl[j].
Softmaxes: softmax1 uses true max; chain softmaxes use fixed offset
exp(100*v - 100) (safe: max component >= 1/16); output softmax uses the exact
max Mh+Ml as per-partition activation bias.

Sharding: pure data parallel over batch, 8 cores x 4096 rows.
"""

import numpy as np

B_FULL = 32768
N_CORES = 8
B_CORE = B_FULL // N_CORES  # 4096
P = 128
NT = B_CORE // P  # 32 tiles

_BUILT = None


def _build():
    import concourse.bass as bass
    import concourse.bacc as bacc
    import concourse.mybir as mybir
    import concourse.tile as tile

    f32 = mybir.dt.float32
    AF = mybir.ActivationFunctionType
    AX = mybir.AxisListType
    OP = mybir.AluOpType

    nc = bacc.Bacc("TRN2", target_bir_lowering=False, debug=False)
    a_d = nc.dram_tensor("a", [B_CORE, 4, 256], f32, kind="ExternalInput")
    b_d = nc.dram_tensor("b", [B_CORE, 4, 256], f32, kind="ExternalInput")
    out_d = nc.dram_tensor("out", [B_CORE, 4, 256], f32, kind="ExternalOutput")

    def rawap(base_ap, off_elems, dims):
        # dims: list of [step, count] free dims; keeps base partition dim
        part = base_ap.ap[0]
        return bass.AP(base_ap.tensor, base_ap.offset + off_elems, [list(part)] + [list(d) for d in dims])

    with tile.TileContext(nc) as tc:
        with (
            tc.tile_pool(name="persist", bufs=1) as pp,
            tc.tile_pool(name="pin", bufs=3) as pin,
            tc.tile_pool(name="pmid", bufs=2) as pmid,
            tc.tile_pool(name="psm", bufs=2) as psm,
            tc.tile_pool(name="pc", bufs=2) as pc,
            tc.tile_pool(name="pcs", bufs=2) as pcs,
        ):
            # ---- persistent tensors ----
            ucat = pp.tile([P, 8, 2, 32, 18], f32, tag="ucat")  # [stage, half, blk, 18]
            A_all = pp.tile([P, 8, 32, 16], f32, tag="A_all")
            carry = pp.tile([P, 32, 2], f32, tag="carry")
            sgn = pp.tile([P, 2], f32, tag="sgn")
            nb100 = pp.tile([P, 1], f32, tag="nb100")
            w_t0 = pp.tile([P, 8, 16, 32], f32, tag="w0")
            w_t1 = pp.tile([P, 8, 16, 32], f32, tag="w1")
            z_t0 = pp.tile([P, 8, 32], f32, tag="z0")
            z_t1 = pp.tile([P, 8, 32], f32, tag="z1")
            w_bufs = [w_t0, w_t1]
            z_bufs = [z_t0, z_t1]
            # chain scratch (serial reuse)
            P_t = pp.tile([P, 2, 32, 18], f32, tag="P_t")
            st_t = pp.tile([P, 32, 18], f32, tag="st")
            e_t = pp.tile([P, 32, 18], f32, tag="e")
            ns_t = pp.tile([P, 32], f32, tag="ns")
            ncr_t = pp.tile([P, 32], f32, tag="ncr")
            rs_t = pp.tile([P, 32], f32, tag="rs")
            rc_t = pp.tile([P, 32], f32, tag="rc")

            nc.gpsimd.memset(nb100[:], -100.0)
            nc.gpsimd.memset(sgn[:, 0:1], -1.0)
            nc.gpsimd.memset(sgn[:, 1:2], 1.0)
            for k in range(2):
                nc.gpsimd.memset(w_bufs[k][:, :, :, 16:32], 0.0)
                nc.gpsimd.memset(z_bufs[k][:, :, 31:32], 0.0)
            nc.gpsimd.memset(carry[:, :, 0:1], 1.0)
            nc.gpsimd.memset(carry[:, :, 1:2], 0.0)

            a_v = a_d.ap().rearrange("(n p) f g -> n p (f g)", p=P)
            b_v = b_d.ap().rearrange("(n p) f g -> n p (f g)", p=P)
            o_v = out_d.ap().rearrange("(n p) f g -> n p (f g)", p=P)

            # ================= Phase A: per-tile nibble dists + conv =========
            for i in range(NT):
                w_t = w_bufs[i % 2]
                z_t = z_bufs[i % 2]
                a_t = pin.tile([P, 1024], f32, tag="a")
                b_t = pin.tile([P, 1024], f32, tag="b")
                nc.sync.dma_start(a_t[:], a_v[i])
                nc.sync.dma_start(b_t[:], b_v[i])

                # nibble sums -> c_all [16 groups x 16]; group order:
                # a: (lo p0, hi p0, lo p1, hi p1, ...) = groups 0..7, b: groups 8..15
                c_all = pmid.tile([P, 256], f32, tag="c_all")
                for src, base in ((a_t, 0), (b_t, 128)):
                    hi_in = src[:].rearrange("p (x h l) -> p x h l", x=4, h=16, l=16)
                    # high: sum over low nibble (contiguous innermost)
                    nc.vector.tensor_reduce(
                        rawap(c_all[:], base + 16, [[32, 4], [1, 16]]),
                        hi_in, axis=AX.X, op=OP.add)
                    # low: sum over high nibble (innermost step 16)
                    lo_in = rawap(src[:], 0, [[256, 4], [1, 16], [16, 16]])
                    nc.vector.tensor_reduce(
                        rawap(c_all[:], base + 0, [[32, 4], [1, 16]]),
                        lo_in, axis=AX.X, op=OP.add)

                # softmax over each of the 16 groups
                m16 = psm.tile([P, 16], f32, tag="m16")
                cg = c_all[:].rearrange("p (g e) -> p g e", g=16)
                nc.vector.tensor_reduce(m16[:], cg, axis=AX.X, op=OP.max)
                ts = pmid.tile([P, 256], f32, tag="tsub")
                nc.gpsimd.tensor_sub(
                    ts[:].rearrange("p (g e) -> p g e", g=16), cg,
                    m16[:].unsqueeze(2).broadcast_to([P, 16, 16]))
                te = pmid.tile([P, 256], f32, tag="texp")
                nc.scalar.activation(te[:], ts[:], AF.Exp, scale=100.0)
                n16 = psm.tile([P, 16], f32, tag="n16")
                nc.vector.tensor_reduce(
                    n16[:], te[:].rearrange("p (g e) -> p g e", g=16),
                    axis=AX.X, op=OP.add)
                r16 = psm.tile([P, 16], f32, tag="r16")
                nc.vector.reciprocal(r16[:], n16[:])
                x_t = pmid.tile([P, 256], f32, tag="x_t")
                nc.gpsimd.tensor_mul(
                    x_t[:].rearrange("p (g e) -> p g e", g=16),
                    te[:].rearrange("p (g e) -> p g e", g=16),
                    r16[:].unsqueeze(2).broadcast_to([P, 16, 16]))

                # outer products w[s,i,j] = xa[s,i]*xb[s,j] (cols 0..15; 16..31 stay 0)
                xa = rawap(x_t[:], 0, [[16, 8], [1, 16], [0, 16]])
                xb = rawap(x_t[:], 128, [[16, 8], [0, 16], [1, 16]])
                nc.gpsimd.tensor_mul(w_t[:, :, :, 0:16], xa, xb)

                # z[s,t] = sum_i w[s, i, t-i]  (antidiagonal, flat stride 31)
                nc.vector.tensor_reduce(
                    rawap(z_t[:], 0, [[32, 8], [1, 31]]),
                    rawap(w_t[:], 0, [[512, 8], [1, 31], [31, 16]]),
                    axis=AX.X, op=OP.add)

                # zz[s, half] = sum z[s, 16*half : 16*half+16]
                zz = psm.tile([P, 16], f32, tag="zz")
                nc.vector.tensor_reduce(
                    zz[:].rearrange("p (s h) -> p s h", s=8),
                    z_t[:].rearrange("p s (h e) -> p s h e", h=2),
                    axis=AX.X, op=OP.add)

                # u = z[:,0:16] + z[:,16:32] -> ucat[:, s, 0, i, 0:16]
                nc.vector.tensor_add(
                    rawap(ucat[:], 18 * i, [[1152, 8], [1, 16]]),
                    z_t[:, :, 0:16], z_t[:, :, 16:32])
                # rot half: rot[l] = u[l-1]; rot[0] = u[15]  (ACT copies: DVE is scarce)
                nc.scalar.copy(
                    rawap(ucat[:], 576 + 18 * i + 1, [[1152, 8], [1, 15]]),
                    rawap(ucat[:], 18 * i, [[1152, 8], [1, 15]]))
                nc.scalar.copy(
                    rawap(ucat[:], 576 + 18 * i, [[1152, 8], [1, 1]]),
                    rawap(ucat[:], 18 * i + 15, [[1152, 8], [1, 1]]))
                # main extras: [Z0, Z1]
                nc.scalar.copy(
                    rawap(ucat[:], 18 * i + 16, [[1152, 8], [1, 2]]),
                    zz[:].rearrange("p (s h) -> p s h", s=8))
                # rot extras: [Z0 - z15, Z1 + z15] = zz + z15*[-1, +1]
                zs = psm.tile([P, 16], f32, tag="zs")
                nc.vector.tensor_mul(
                    zs[:].rearrange("p (s h) -> p s h", s=8),
                    rawap(z_t[:], 15, [[32, 8], [0, 2]]),
                    sgn[:].unsqueeze(1).broadcast_to([P, 8, 2]))
                nc.vector.tensor_add(
                    rawap(ucat[:], 576 + 18 * i + 16, [[1152, 8], [1, 2]]),
                    zz[:].rearrange("p (s h) -> p s h", s=8),
                    zs[:].rearrange("p (s h) -> p s h", s=8))

            # ================= Phase B: serial carry chain ===================
            for s in range(8):
                # P = ucat[s] * carry  (main*c0, rot*c1)
                nc.vector.tensor_mul(
                    P_t[:], ucat[:, s],
                    rawap(carry[:], 0, [[1, 2], [2, 32], [0, 18]]))
                nc.vector.tensor_add(st_t[:], P_t[:, 0], P_t[:, 1])
                nc.scalar.activation(e_t[:], st_t[:], AF.Exp, bias=nb100[:], scale=100.0)
                nc.vector.tensor_reduce(ns_t[:], e_t[:, :, 0:16], axis=AX.X, op=OP.add)
                nc.vector.tensor_reduce(ncr_t[:], e_t[:, :, 16:18], axis=AX.X, op=OP.add)
                nc.vector.reciprocal(rs_t[:], ns_t[:])
                nc.vector.reciprocal(rc_t[:], ncr_t[:])
                # new carry
                nc.vector.tensor_mul(
                    rawap(carry[:], 0, [[2, 32], [1, 2]]),
                    e_t[:, :, 16:18],
                    rc_t[:].unsqueeze(2).broadcast_to([P, 32, 2]))
                # normalized s-dist
                nc.vector.tensor_mul(
                    A_all[:, s], e_t[:, :, 0:16],
                    rs_t[:].unsqueeze(2).broadcast_to([P, 32, 16]))

            # ================= Phase C: output softmax =======================
            for i in range(NT):
                M8 = pcs.tile([P, 8], f32, tag="M8")
                nc.vector.tensor_reduce(
                    M8[:],
                    rawap(A_all[:], 16 * i, [[512, 8], [1, 16]]),
                    axis=AX.X, op=OP.max)
                Ms4 = pcs.tile([P, 4], f32, tag="Ms4")
                nc.vector.tensor_add(
                    Ms4[:],
                    rawap(M8[:], 0, [[2, 4]]),
                    rawap(M8[:], 1, [[2, 4]]))
                nb4 = pcs.tile([P, 4], f32, tag="nb4")
                nc.vector.tensor_scalar_mul(nb4[:], Ms4[:], -100.0)
                o_t = pc.tile([P, 4, 16, 16], f32, tag="o_t")
                # o[p, pos, ih, jl] = sh[pos, ih] + sl[pos, jl]  (on GpSimd: DVE is scarce)
                nc.gpsimd.tensor_add(
                    o_t[:],
                    rawap(A_all[:], 512 + 16 * i, [[1024, 4], [1, 16], [0, 16]]),
                    rawap(A_all[:], 16 * i, [[1024, 4], [0, 16], [1, 16]]))
                no4 = pcs.tile([P, 4], f32, tag="no4")
                for p4 in range(4):
                    nc.scalar.activation(
                        o_t[:, p4], o_t[:, p4], AF.Exp,
                        bias=nb4[:, p4:p4 + 1], scale=100.0,
                        accum_out=no4[:, p4:p4 + 1])
                ro4 = pcs.tile([P, 4], f32, tag="ro4")
                nc.vector.reciprocal(ro4[:], no4[:])
                for p4 in range(4):
                    nc.vector.tensor_scalar_mul(o_t[:, p4], o_t[:, p4], ro4[:, p4:p4 + 1])
                nc.sync.dma_start(o_v[i], o_t[:].rearrange("p a b c -> p (a b c)"))

    nc.compile()
    return nc


def _get_nc():
    global _BUILT
    if _BUILT is None:
        _BUILT = _build()
    return _BUILT


def kernel(a, b, add_table=None, carry_table=None, b2n=None, n2b=None, **_kw):
    from concourse.bass_utils import run_bass_kernel_spmd

    a = np.ascontiguousarray(np.asarray(a, dtype=np.float32))
    b = np.ascontiguousarray(np.asarray(b, dtype=np.float32))
    nc = _get_nc()
    in_maps = [
        {"a": a[i * B_CORE:(i + 1) * B_CORE], "b": b[i * B_CORE:(i + 1) * B_CORE]}
        for i in range(N_CORES)
    ]
    res = run_bass_kernel_spmd(nc, in_maps, core_ids=list(range(N_CORES)))
    out = np.concatenate([r["out"] for r in res.results], axis=0)
    return out.astype(np.float32)



# revision 10
# speedup vs baseline: 2.0513x; 2.0513x over previous
"""Trainium2 Bass kernel for nn_MoEALU (soft ripple-carry byte adder), v2.

Restructured math (validated in sim.py against the jax reference):
  - nibble sums: segmented sums of the 256-wide byte distribution per pos.
  - softmax1 kept UNNORMALIZED (te = exp(100(c - max))); the normalizer
    kappa_s = 1/(sum te_a * sum te_b) = 1/sum_m u_raw[m] is folded into the
    17 conv outputs afterwards.
  - cyclic conv u[m] = sum_i xa_i xb_{(m-i)%16} via a doubled-xb buffer
    (stride [+1 m, -1 i] AP); z15 == u[15].
  - Z1 = 1 - sum_i xa_i p[15-i] where p = prefix sums of xb (one DVE scan).
  - carry chain: softmax over 2 == sigmoid => gamma' = sig(100(d + e*gamma)),
    d = Z1-Z0 = 1-2W, e = 2*z15. At temp 100 gamma saturates to {0,1}, so the
    recurrence linearizes EXACTLY (validated): gamma' = v0 + (v1-v0)*gamma
    with v0 = sig(100 d), v1 = sig(100(d+e)) => one tensor_tensor_scan with
    per-tile reset via zeroed b-coefficient.
  - s-logits: s = u + (rot1(u) - u)*gamma_in.
  - output softmax factorizes: softmax_256(100(sh_i + sl_j)) =
    softmax_16(100 sh) (x) softmax_16(100 sl): two 16-wide softmaxes and an
    outer product per byte; chain softmaxes use the fixed offset exp(100v-100)
    (max component >= 1/16 so the top stays >= e^-93.75 > 0 in fp32).
fp16 used for: te / conv products / u storage / A / final outer + DMA-out
(all validated <= ~1e-3 L2). exp outputs eh/e2 stay fp32 (fp16 underflows
for near-flat dists).

Sharding: pure data parallel over batch, 8 cores x 4096 rows.
"""

import numpy as np

B_FULL = 32768
N_CORES = 8
B_CORE = B_FULL // N_CORES  # 4096
P = 128
NT = B_CORE // P  # 32 tiles

_BUILT = None


def _build():
    import concourse.bass as bass
    import concourse.bacc as bacc
    import concourse.mybir as mybir
    import concourse.tile as tile

    f32 = mybir.dt.float32
    f16 = mybir.dt.float16
    AF = mybir.ActivationFunctionType
    AX = mybir.AxisListType
    OP = mybir.AluOpType

    nc = bacc.Bacc("TRN2", target_bir_lowering=False, debug=False)
    a_d = nc.dram_tensor("a", [B_CORE, 4, 256], f32, kind="ExternalInput")
    b_d = nc.dram_tensor("b", [B_CORE, 4, 256], f32, kind="ExternalInput")
    out_d = nc.dram_tensor("out", [B_CORE, 4, 256], f16, kind="ExternalOutput")

    def ap(base_ap, off, dims):
        part = base_ap.ap[0]
        return bass.AP(base_ap.tensor, base_ap.offset + off,
                       [list(part)] + [list(d) for d in dims])

    with tile.TileContext(nc) as tc:
        with (
            tc.tile_pool(name="persist", bufs=1) as pp,
            tc.tile_pool(name="pin", bufs=3) as pin,
            tc.tile_pool(name="pa", bufs=2) as pa,
            tc.tile_pool(name="prep", bufs=2) as prep,
            tc.tile_pool(name="pout", bufs=3) as pout,
        ):
            # ---------------- persistent tensors ----------------
            u_all = pp.tile([P, NT, 8, 18], f16, tag="u_all")
            d_all = pp.tile([P, NT, 8], f32, tag="d_all")
            e_all = pp.tile([P, NT, 8], f32, tag="e_all")
            dpe = pp.tile([P, NT, 8], f32, tag="dpe")
            v0 = pp.tile([P, NT, 8], f32, tag="v0")
            v1 = pp.tile([P, NT, 8], f32, tag="v1")
            bco = pp.tile([P, NT, 8], f32, tag="bco")
            gg = pp.tile([P, 257], f32, tag="gg")
            g16 = pp.tile([P, NT, 8, 16], f16, tag="g16")
            # phase C (single-use, whole batch)
            dlt = pp.tile([P, NT, 8, 16], f16, tag="dlt")
            tb = pp.tile([P, NT, 8, 16], f16, tag="tb")
            sb = pp.tile([P, NT, 8, 16], f16, tag="sb")
            eh = pp.tile([P, NT, 8, 16], f32, tag="eh")
            ns = pp.tile([P, NT, 8], f32, tag="ns")
            r1 = pp.tile([P, NT, 8], f32, tag="r1")
            A16 = pp.tile([P, NT, 8, 16], f16, tag="A16")
            e2 = pp.tile([P, NT, 8, 16], f32, tag="e2")
            s2 = pp.tile([P, NT, 8], f32, tag="s2")
            r2 = pp.tile([P, NT, 8], f32, tag="r2")
            e2h = pp.tile([P, NT, 4, 16], f16, tag="e2h")
            e2l = pp.tile([P, NT, 4, 16], f16, tag="e2l")

            nb100 = pp.tile([P, 1], f32, tag="nb100")
            nc.gpsimd.memset(nb100[:], -100.0)
            # per-tile reset slots for the chain scan
            nc.gpsimd.memset(ap(bco[:], 0, [[8, NT]]), 0.0)

            a_v = a_d.ap().rearrange("(n p) f g -> n p (f g)", p=P)
            b_v = b_d.ap().rearrange("(n p) f g -> n p (f g)", p=P)
            o_v = out_d.ap().rearrange("(n p) f g -> n p (f g)", p=P)

            # ================= Phase A: per-tile =================
            for i in range(NT):
                ab = pin.tile([P, 2048], f32, tag="ab")
                nc.sync.dma_start(ap(ab[:], 0, [[1, 1024]]), a_v[i])
                nc.sync.dma_start(ap(ab[:], 1024, [[1, 1024]]), b_v[i])

                # c_all [2 tensors, 8 stages(lo0,hi0,..), 16 bins]
                c_all = pa.tile([P, 256], f32, tag="c_all")
                # hi sums (contiguous l) -> odd stages [DVE reduce]
                nc.vector.tensor_reduce(
                    ap(c_all[:], 16, [[128, 2], [32, 4], [1, 16]]),
                    ap(ab[:], 0, [[1024, 2], [256, 4], [16, 16], [1, 16]]),
                    axis=AX.X, op=OP.add)
                # lo sums via TT tree on Pool -> even stages
                lo1 = pa.tile([P, 2, 4, 16, 8], f32, tag="lo1")
                nc.gpsimd.tensor_add(
                    lo1[:],
                    ap(ab[:], 0, [[1024, 2], [256, 4], [1, 16], [16, 8]]),
                    ap(ab[:], 128, [[1024, 2], [256, 4], [1, 16], [16, 8]]))
                lo2 = pa.tile([P, 2, 4, 16, 4], f32, tag="lo2")
                nc.gpsimd.tensor_add(
                    lo2[:],
                    ap(lo1[:], 0, [[512, 2], [128, 4], [8, 16], [1, 4]]),
                    ap(lo1[:], 4, [[512, 2], [128, 4], [8, 16], [1, 4]]))
                lo3 = pa.tile([P, 2, 4, 16, 2], f32, tag="lo3")
                nc.gpsimd.tensor_add(
                    lo3[:],
                    ap(lo2[:], 0, [[256, 2], [64, 4], [4, 16], [1, 2]]),
                    ap(lo2[:], 2, [[256, 2], [64, 4], [4, 16], [1, 2]]))
                nc.gpsimd.tensor_add(
                    ap(c_all[:], 0, [[128, 2], [32, 4], [1, 16]]),
                    ap(lo3[:], 0, [[128, 2], [32, 4], [2, 16]]),
                    ap(lo3[:], 1, [[128, 2], [32, 4], [2, 16]]))

                # softmax1: true max per 16-group, subtract, exp -> fp16 te
                m16 = pa.tile([P, 16], f32, tag="m16")
                nc.vector.tensor_reduce(
                    m16[:], c_all[:].rearrange("p (g e) -> p g e", g=16),
                    axis=AX.X, op=OP.max)
                ts = pa.tile([P, 256], f32, tag="ts")
                nc.gpsimd.tensor_sub(
                    ts[:].rearrange("p (g e) -> p g e", g=16),
                    c_all[:].rearrange("p (g e) -> p g e", g=16),
                    ap(m16[:], 0, [[1, 16], [0, 16]]))
                # T: [2, 8, 16] contiguous te; xbd: b doubled along bins
                T = pa.tile([P, 2, 8, 16], f16, tag="T")
                nc.scalar.activation(
                    T[:].rearrange("p a b c -> p (a b c)"),
                    ts[:], AF.Exp, scale=100.0)
                xbd = pa.tile([P, 8, 32], f16, tag="xbd")
                nc.scalar.copy(
                    ap(xbd[:], 0, [[32, 8], [1, 16]]),
                    ap(T[:], 128, [[16, 8], [1, 16]]))
                nc.scalar.copy(
                    ap(xbd[:], 16, [[32, 8], [1, 16]]),
                    ap(T[:], 128, [[16, 8], [1, 16]]))

                # prefix sums of te_b: S[0]=0, S[1..128] = scan
                S = pa.tile([P, 132], f32, tag="S")
                nc.gpsimd.memset(ap(S[:], 0, [[1, 1]]), 0.0)
                nc.vector.tensor_tensor_scan(
                    ap(S[:], 1, [[1, 128]]),
                    ap(T[:], 128, [[1, 128]]),
                    ap(T[:], 128, [[1, 128]]),
                    0.0, OP.add, OP.bypass)
                p16 = pa.tile([P, 8, 16], f16, tag="p16")
                nc.gpsimd.tensor_sub(
                    p16[:],
                    ap(S[:], 1, [[16, 8], [1, 16]]),
                    ap(S[:], 0, [[16, 8], [0, 16]]))

                # conv products q[s, c, i]: c=0..15 cyclic cols, c=16 W col
                q = pa.tile([P, 8, 17, 16], f16, tag="q")
                nc.vector.tensor_mul(
                    ap(q[:], 0, [[272, 8], [16, 16], [1, 16]]),
                    ap(T[:], 0, [[16, 8], [0, 16], [1, 16]]),
                    ap(xbd[:], 16, [[32, 8], [1, 16], [-1, 16]]))
                nc.vector.tensor_mul(
                    ap(q[:], 256, [[272, 8], [1, 16]]),
                    ap(T[:], 0, [[16, 8], [1, 16]]),
                    ap(p16[:], 15, [[16, 8], [-1, 16]]))

                # i-reduction tree (fp16 2x adds) -> qr [8, 17]
                qt1 = pa.tile([P, 8, 17, 8], f16, tag="qt1")
                nc.vector.tensor_add(
                    qt1[:],
                    ap(q[:], 0, [[272, 8], [16, 17], [1, 8]]),
                    ap(q[:], 8, [[272, 8], [16, 17], [1, 8]]))
                qt2 = pa.tile([P, 8, 17, 4], f16, tag="qt2")
                nc.vector.tensor_add(
                    qt2[:],
                    ap(qt1[:], 0, [[136, 8], [8, 17], [1, 4]]),
                    ap(qt1[:], 4, [[136, 8], [8, 17], [1, 4]]))
                qt3 = pa.tile([P, 8, 17, 2], f16, tag="qt3")
                nc.vector.tensor_add(
                    qt3[:],
                    ap(qt2[:], 0, [[68, 8], [4, 17], [1, 2]]),
                    ap(qt2[:], 2, [[68, 8], [4, 17], [1, 2]]))
                qr = pa.tile([P, 8, 17], f16, tag="qr")
                nc.vector.tensor_add(
                    qr[:],
                    ap(qt3[:], 0, [[34, 8], [2, 17]]),
                    ap(qt3[:], 1, [[34, 8], [2, 17]]))

                # kappa = 1/sum_m u_raw
                Su = pa.tile([P, 8], f32, tag="Su")
                nc.vector.tensor_reduce(
                    Su[:], ap(qr[:], 0, [[17, 8], [1, 16]]),
                    axis=AX.X, op=OP.add)
                rk = pa.tile([P, 8], f32, tag="rk")
                nc.vector.reciprocal(rk[:], Su[:])

                # normalized u -> u_all[i] slots 1..16; wrap u15 -> slot 0
                nc.gpsimd.tensor_mul(
                    ap(u_all[:], 144 * i + 1, [[18, 8], [1, 16]]),
                    ap(qr[:], 0, [[17, 8], [1, 16]]),
                    ap(rk[:], 0, [[1, 8], [0, 16]]))
                nc.gpsimd.tensor_mul(
                    ap(u_all[:], 144 * i, [[18, 8]]),
                    ap(qr[:], 15, [[17, 8]]),
                    ap(rk[:], 0, [[1, 8]]))
                # d = 1 - 2*W*rk ; e = 2*u15n
                t8 = pa.tile([P, 8], f32, tag="t8")
                nc.gpsimd.tensor_mul(
                    t8[:], ap(qr[:], 16, [[17, 8]]), ap(rk[:], 0, [[1, 8]]))
                nc.gpsimd.tensor_scalar(
                    ap(d_all[:], 8 * i, [[1, 8]]), t8[:],
                    -2.0, 1.0, op0=OP.mult, op1=OP.add)
                nc.gpsimd.tensor_scalar_mul(
                    ap(e_all[:], 8 * i, [[1, 8]]),
                    ap(u_all[:], 144 * i, [[18, 8]]), 2.0)

            # ================= Chain: linearized scan =================
            nc.vector.tensor_add(dpe[:], d_all[:], e_all[:])
            nc.scalar.activation(v0[:], d_all[:], AF.Sigmoid, scale=100.0)
            nc.scalar.activation(v1[:], dpe[:], AF.Sigmoid, scale=100.0)
            nc.vector.tensor_sub(
                ap(bco[:], 1, [[8, NT], [1, 7]]),
                ap(v1[:], 1, [[8, NT], [1, 7]]),
                ap(v0[:], 1, [[8, NT], [1, 7]]))
            nc.vector.tensor_tensor_scan(
                ap(gg[:], 1, [[1, 256]]),
                bco[:].rearrange("p a b -> p (a b)"),
                v0[:].rearrange("p a b -> p (a b)"),
                0.0, OP.mult, OP.add)
            # gg[8t] slots hold only discarded stage-7 carry-outs; zero them
            # so gg[8t+s] == gamma_in(tile t, stage s) with gamma_in(t,0)=0.
            nc.gpsimd.memset(ap(gg[:], 0, [[8, NT]]), 0.0)
            # gamma_in broadcast to 16 bins (fp16)
            nc.scalar.copy(
                g16[:], ap(gg[:], 0, [[8, NT], [1, 8], [0, 16]]))

            # ================= Phase C: batched =================
            u_sl = ap(u_all[:], 1, [[144, NT], [18, 8], [1, 16]])
            rot_sl = ap(u_all[:], 0, [[144, NT], [18, 8], [1, 16]])
            nc.vector.tensor_sub(dlt[:], rot_sl, u_sl)
            nc.vector.tensor_mul(tb[:], dlt[:], g16[:])
            nc.vector.tensor_add(sb[:], u_sl, tb[:])
            nc.scalar.activation(eh[:], sb[:], AF.Exp, bias=nb100[:], scale=100.0)
            nc.vector.tensor_reduce(
                ns[:], eh[:].rearrange("p a b e -> p (a b) e"),
                axis=AX.X, op=OP.add)
            nc.vector.reciprocal(r1[:], ns[:])
            nc.vector.tensor_mul(
                A16[:], eh[:],
                ap(r1[:], 0, [[8, NT], [1, 8], [0, 16]]))
            nc.scalar.activation(e2[:], A16[:], AF.Exp, bias=nb100[:], scale=100.0)
            nc.vector.tensor_reduce(
                s2[:], e2[:].rearrange("p a b e -> p (a b) e"),
                axis=AX.X, op=OP.add)
            nc.vector.reciprocal(r2[:], s2[:])
            # normalize each side separately (r2h*r2l overflows fp32)
            nc.vector.tensor_mul(
                e2h[:],
                ap(e2[:], 16, [[128, NT], [32, 4], [1, 16]]),
                ap(r2[:], 1, [[8, NT], [2, 4], [0, 16]]))
            nc.vector.tensor_mul(
                e2l[:],
                ap(e2[:], 0, [[128, NT], [32, 4], [1, 16]]),
                ap(r2[:], 0, [[8, NT], [2, 4], [0, 16]]))

            # outer products + store, per 4-tile replication groups
            for g in range(NT // 4):
                rep = prep.tile([P, 4, 4, 16, 16], f16, tag="rep")
                nc.scalar.copy(
                    rep[:],
                    ap(e2h[:], 256 * g, [[64, 4], [16, 4], [1, 16], [0, 16]]))
                for t in range(4):
                    i = 4 * g + t
                    o_t = pout.tile([P, 4, 16, 16], f16, tag="o_t")
                    nc.vector.tensor_mul(
                        o_t[:],
                        ap(rep[:], 1024 * t, [[256, 4], [16, 16], [1, 16]]),
                        ap(e2l[:], 64 * i, [[16, 4], [0, 16], [1, 16]]))
                    nc.sync.dma_start(
                        o_v[i], o_t[:].rearrange("p a b c -> p (a b c)"))

    nc.compile()
    return nc


def _get_nc():
    global _BUILT
    if _BUILT is None:
        _BUILT = _build()
    return _BUILT


def kernel(a, b, add_table=None, carry_table=None, b2n=None, n2b=None, **_kw):
    from concourse.bass_utils import run_bass_kernel_spmd

    a = np.ascontiguousarray(np.asarray(a, dtype=np.float32))
    b = np.ascontiguousarray(np.asarray(b, dtype=np.float32))
    nc = _get_nc()
    in_maps = [
        {"a": a[i * B_CORE:(i + 1) * B_CORE], "b": b[i * B_CORE:(i + 1) * B_CORE]}
        for i in range(N_CORES)
    ]
    res = run_bass_kernel_spmd(nc, in_maps, core_ids=list(range(N_CORES)))
    out = np.concatenate([r["out"] for r in res.results], axis=0)
    return out.astype(np.float32)


# revision 29
# speedup vs baseline: 2.2950x; 1.1188x over previous
"""Trainium2 Bass kernel for nn_MoEALU (soft ripple-carry byte adder), v3.

Restructured math (validated in sim.py against the jax reference):
  - nibble sums: segmented sums of the 256-wide byte distribution per pos.
  - softmax1 kept UNNORMALIZED (te = exp(100(c - max))); the normalizer
    kappa_s = 1/(sum te_a * sum te_b) = 1/sum_m u_raw[m] is folded into the
    17 conv outputs afterwards.
  - cyclic conv u[m] = sum_i xa_i xb_{(m-i)%16} via a doubled-xb buffer
    (stride [+1 m, -1 i] AP); z15 == u[15].
  - Z1 = 1 - sum_i xa_i p[15-i] where p = prefix sums of xb (one scan).
  - carry chain: softmax over 2 == sigmoid => gamma' = sig(100(d + e*gamma)),
    d = Z1-Z0 = 1-2W, e = 2*z15. At temp 100 gamma saturates to {0,1}, so the
    recurrence linearizes EXACTLY (validated): gamma' = v0 + (v1-v0)*gamma
    with v0 = sig(100 d), v1 = sig(100(d+e)) => one tensor_tensor_scan with
    per-tile reset via zeroed b-coefficient.
  - s-logits: s = u + (rot1(u) - u)*gamma_in.
  - output softmax factorizes: softmax_256(100(sh_i + sl_j)) =
    softmax_16(100 sh) (x) softmax_16(100 sl): two 16-wide softmaxes and an
    outer product per byte; chain softmaxes use the fixed offset exp(100v-100).
fp16: te / conv products / u storage / A / final outer + DMA-out. exp outputs
eh/e2 stay fp32 (fp16 underflows for near-flat dists); per-side r2 folds
(r2h*r2l overflows fp32).

Schedule: phase A software-pipelined per tile; carry chain + phase C run per
16-tile half, with phase C emitted in 8-tile chunks x 4 stages interleaved
into the phase-A stream so DVE never stalls on ACT exps.

Sharding: pure data parallel over batch, 8 cores x 4096 rows.
"""

import numpy as np

B_FULL = 32768
N_CORES = 8
B_CORE = B_FULL // N_CORES  # 4096
P = 128
NT = B_CORE // P  # 32 tiles
HT = NT // 2  # tiles per half
CT = 8  # tiles per phase-C chunk

_BUILT = None


def _build():
    import concourse.bass as bass
    import concourse.bacc as bacc
    import concourse.mybir as mybir
    import concourse.tile as tile

    f32 = mybir.dt.float32
    f16 = mybir.dt.float16
    AF = mybir.ActivationFunctionType
    AX = mybir.AxisListType
    OP = mybir.AluOpType

    nc = bacc.Bacc("TRN2", target_bir_lowering=False, debug=False)
    a_d = nc.dram_tensor("a", [B_CORE, 4, 256], f32, kind="ExternalInput")
    b_d = nc.dram_tensor("b", [B_CORE, 4, 256], f32, kind="ExternalInput")
    out_d = nc.dram_tensor("out", [B_CORE, 4, 256], f16, kind="ExternalOutput")

    def ap(base_ap, off, dims):
        part = base_ap.ap[0]
        return bass.AP(base_ap.tensor, base_ap.offset + off,
                       [list(part)] + [list(d) for d in dims])

    with tile.TileContext(nc) as tc:
        with (
            tc.tile_pool(name="persist", bufs=1) as pp,
            tc.tile_pool(name="pin", bufs=2) as pin,
            tc.tile_pool(name="pa", bufs=2) as pa,
            tc.tile_pool(name="pa1", bufs=2) as pa1,
            tc.tile_pool(name="pc", bufs=2) as pc,
            tc.tile_pool(name="prep", bufs=2) as prep,
            tc.tile_pool(name="pout", bufs=3) as pout,
        ):
            # ---------------- persistent tensors ----------------
            u_all = pp.tile([P, NT, 8, 18], f16, tag="u_all")
            d_all = pp.tile([P, NT, 8], f32, tag="d_all")
            e_all = pp.tile([P, NT, 8], f32, tag="e_all")
            dpe = pp.tile([P, NT, 8], f32, tag="dpe")
            v0 = pp.tile([P, NT, 8], f32, tag="v0")
            v1 = pp.tile([P, NT, 8], f32, tag="v1")
            bco = pp.tile([P, NT, 8], f32, tag="bco")
            gg = pp.tile([P, 257], f32, tag="gg")
            g16 = pp.tile([P, NT, 8, 16], f16, tag="g16")
            nb100 = pp.tile([P, 1], f32, tag="nb100")
            nc.gpsimd.memset(nb100[:], -100.0)
            nc.gpsimd.memset(ap(bco[:], 0, [[8, NT]]), 0.0)

            a_v = a_d.ap().rearrange("(n p) f g -> n p (f g)", p=P)
            b_v = b_d.ap().rearrange("(n p) f g -> n p (f g)", p=P)
            o_v = out_d.ap().rearrange("(n p) f g -> n p (f g)", p=P)

            # ----- phase A stage 1 (2-tile super-tile): load+sums+max ----
            # ab layout [tensor, t, 1024] so (tensor,t) merges to stride 1024
            def phase_a1(u):
                ab = pin.tile([P, 2, 2, 1024], f32, tag="ab")
                for t in range(2):
                    nc.sync.dma_start(
                        ap(ab[:], 1024 * t, [[1, 1024]]), a_v[2 * u + t])
                    nc.sync.dma_start(
                        ap(ab[:], 2048 + 1024 * t, [[1, 1024]]), b_v[2 * u + t])
                # c_all [tensor, t, 8 stages, 16]
                c_all = pa1.tile([P, 512], f32, tag="c_all")
                nc.vector.tensor_reduce(
                    ap(c_all[:], 16, [[128, 4], [32, 4], [1, 16]]),
                    ap(ab[:], 0, [[1024, 4], [256, 4], [16, 16], [1, 16]]),
                    axis=AX.X, op=OP.add)
                lo1 = pa.tile([P, 4, 4, 16, 8], f32, tag="lo1")
                nc.gpsimd.tensor_add(
                    lo1[:],
                    ap(ab[:], 0, [[1024, 4], [256, 4], [1, 16], [16, 8]]),
                    ap(ab[:], 128, [[1024, 4], [256, 4], [1, 16], [16, 8]]))
                lo2 = pa.tile([P, 4, 4, 16, 4], f32, tag="lo2")
                nc.gpsimd.tensor_add(
                    lo2[:],
                    ap(lo1[:], 0, [[512, 4], [128, 4], [8, 16], [1, 4]]),
                    ap(lo1[:], 4, [[512, 4], [128, 4], [8, 16], [1, 4]]))
                lo3 = pa.tile([P, 4, 4, 16, 2], f32, tag="lo3")
                nc.gpsimd.tensor_add(
                    lo3[:],
                    ap(lo2[:], 0, [[256, 4], [64, 4], [4, 16], [1, 2]]),
                    ap(lo2[:], 2, [[256, 4], [64, 4], [4, 16], [1, 2]]))
                nc.gpsimd.tensor_add(
                    ap(c_all[:], 0, [[128, 4], [32, 4], [1, 16]]),
                    ap(lo3[:], 0, [[128, 4], [32, 4], [2, 16]]),
                    ap(lo3[:], 1, [[128, 4], [32, 4], [2, 16]]))
                m16 = pa1.tile([P, 32], f32, tag="m16")
                nc.vector.tensor_reduce(
                    m16[:], c_all[:].rearrange("p (g e) -> p g e", g=32),
                    axis=AX.X, op=OP.max)
                return c_all, m16

            # ----- phase A stage 2 (2-tile super-tile) -----
            def phase_a2(u, c_all, m16):
                ts = pa.tile([P, 512], f32, tag="ts")
                nc.gpsimd.tensor_sub(
                    ts[:].rearrange("p (g e) -> p g e", g=32),
                    c_all[:].rearrange("p (g e) -> p g e", g=32),
                    ap(m16[:], 0, [[1, 32], [0, 16]]))
                # T [tensor, t, s, 16]: b-half contiguous at 256 for the scan
                T = pa.tile([P, 2, 2, 8, 16], f16, tag="T")
                nc.scalar.activation(
                    T[:].rearrange("p a b c d -> p (a b c d)"),
                    ts[:], AF.Exp, scale=100.0)
                xbd = pa.tile([P, 2, 8, 32], f16, tag="xbd")
                nc.scalar.copy(
                    ap(xbd[:], 0, [[256, 2], [32, 8], [1, 16]]),
                    ap(T[:], 256, [[128, 2], [16, 8], [1, 16]]))
                nc.scalar.copy(
                    ap(xbd[:], 16, [[256, 2], [32, 8], [1, 16]]),
                    ap(T[:], 256, [[128, 2], [16, 8], [1, 16]]))
                S = pa.tile([P, 260], f32, tag="S")
                nc.gpsimd.memset(ap(S[:], 0, [[1, 1]]), 0.0)
                nc.vector.tensor_tensor_scan(
                    ap(S[:], 1, [[1, 256]]),
                    ap(T[:], 256, [[1, 256]]),
                    ap(T[:], 256, [[1, 256]]),
                    0.0, OP.add, OP.bypass)
                p16 = pa.tile([P, 2, 8, 16], f16, tag="p16")
                nc.vector.tensor_sub(
                    p16[:],
                    ap(S[:], 1, [[16, 16], [1, 16]]),
                    ap(S[:], 0, [[16, 16], [0, 16]]))
                q = pa.tile([P, 2, 8, 17, 16], f16, tag="q")
                nc.vector.tensor_mul(
                    ap(q[:], 0, [[2176, 2], [272, 8], [16, 16], [1, 16]]),
                    ap(T[:], 0, [[128, 2], [16, 8], [0, 16], [1, 16]]),
                    ap(xbd[:], 16, [[256, 2], [32, 8], [1, 16], [-1, 16]]))
                nc.vector.tensor_mul(
                    ap(q[:], 256, [[2176, 2], [272, 8], [1, 16]]),
                    ap(T[:], 0, [[128, 2], [16, 8], [1, 16]]),
                    ap(p16[:], 15, [[128, 2], [16, 8], [-1, 16]]))
                qt1 = pa.tile([P, 2, 8, 17, 8], f16, tag="qt1")
                nc.vector.tensor_add(
                    qt1[:],
                    ap(q[:], 0, [[2176, 2], [272, 8], [16, 17], [1, 8]]),
                    ap(q[:], 8, [[2176, 2], [272, 8], [16, 17], [1, 8]]))
                qt2 = pa.tile([P, 2, 8, 17, 4], f16, tag="qt2")
                nc.vector.tensor_add(
                    qt2[:],
                    ap(qt1[:], 0, [[1088, 2], [136, 8], [8, 17], [1, 4]]),
                    ap(qt1[:], 4, [[1088, 2], [136, 8], [8, 17], [1, 4]]))
                qt3 = pa.tile([P, 2, 8, 17, 2], f16, tag="qt3")
                nc.vector.tensor_add(
                    qt3[:],
                    ap(qt2[:], 0, [[544, 2], [68, 8], [4, 17], [1, 2]]),
                    ap(qt2[:], 2, [[544, 2], [68, 8], [4, 17], [1, 2]]))
                qr = pa.tile([P, 2, 8, 17], f16, tag="qr")
                nc.vector.tensor_add(
                    qr[:],
                    ap(qt3[:], 0, [[272, 2], [34, 8], [2, 17]]),
                    ap(qt3[:], 1, [[272, 2], [34, 8], [2, 17]]))
                Su = pa.tile([P, 16], f32, tag="Su")
                nc.vector.tensor_reduce(
                    Su[:], ap(qr[:], 0, [[17, 16], [1, 16]]),
                    axis=AX.X, op=OP.add)
                rk = pa.tile([P, 16], f32, tag="rk")
                nc.vector.reciprocal(rk[:], Su[:])
                nc.gpsimd.tensor_mul(
                    ap(u_all[:], 288 * u + 1, [[144, 2], [18, 8], [1, 16]]),
                    ap(qr[:], 0, [[136, 2], [17, 8], [1, 16]]),
                    ap(rk[:], 0, [[8, 2], [1, 8], [0, 16]]))
                nc.gpsimd.tensor_mul(
                    ap(u_all[:], 288 * u, [[144, 2], [18, 8]]),
                    ap(qr[:], 15, [[136, 2], [17, 8]]),
                    ap(rk[:], 0, [[8, 2], [1, 8]]))
                t8 = pa.tile([P, 16], f32, tag="t8")
                nc.gpsimd.tensor_mul(
                    t8[:], ap(qr[:], 16, [[136, 2], [17, 8]]),
                    ap(rk[:], 0, [[8, 2], [1, 8]]))
                nc.vector.tensor_scalar(
                    ap(d_all[:], 16 * u, [[1, 16]]), t8[:],
                    -2.0, 1.0, op0=OP.mult, op1=OP.add)
                nc.vector.tensor_scalar_mul(
                    ap(e_all[:], 16 * u, [[1, 16]]),
                    ap(u_all[:], 288 * u, [[144, 2], [18, 8]]), 2.0)

            # --------- carry chain for an 8-tile quarter ---------
            def chain_q(h):
                o = 8 * CT * h  # 64 per quarter
                dsl = ap(d_all[:], o, [[1, 8 * CT]])
                esl = ap(e_all[:], o, [[1, 8 * CT]])
                psl = ap(dpe[:], o, [[1, 8 * CT]])
                v0s = ap(v0[:], o, [[1, 8 * CT]])
                v1s = ap(v1[:], o, [[1, 8 * CT]])
                nc.gpsimd.tensor_add(psl, dsl, esl)
                nc.scalar.activation(v0s, dsl, AF.Sigmoid, scale=100.0)
                nc.scalar.activation(v1s, psl, AF.Sigmoid, scale=100.0)
                nc.gpsimd.tensor_sub(
                    ap(bco[:], o + 1, [[8, CT], [1, 7]]),
                    ap(v1[:], o + 1, [[8, CT], [1, 7]]),
                    ap(v0[:], o + 1, [[8, CT], [1, 7]]))
                nc.vector.tensor_tensor_scan(
                    ap(gg[:], o + 1, [[1, 8 * CT]]),
                    ap(bco[:], o, [[1, 8 * CT]]),
                    ap(v0[:], o, [[1, 8 * CT]]),
                    0.0, OP.mult, OP.add)
                # gg[o+8t] slots hold only discarded stage-7 carry-outs
                nc.gpsimd.memset(ap(gg[:], o, [[8, CT]]), 0.0)
                nc.scalar.copy(
                    ap(g16[:], 128 * CT * h, [[128, CT], [16, 8], [1, 16]]),
                    ap(gg[:], o, [[8, CT], [1, 8], [0, 16]]))

            # --------- phase C chunk stages (CT=8 tiles each) ---------
            def pc_s1(c, tail=False):
                o = 144 * CT * c
                u_sl = ap(u_all[:], o + 1, [[144, CT], [18, 8], [1, 16]])
                rot_sl = ap(u_all[:], o, [[144, CT], [18, 8], [1, 16]])
                dlt = pc.tile([P, CT, 8, 16], f16, tag="dlt")
                tb = pc.tile([P, CT, 8, 16], f16, tag="tb")
                sb = pc.tile([P, CT, 8, 16], f16, tag="sb")
                eh = pc.tile([P, CT, 8, 16], f32, tag="eh")
                nc.vector.tensor_sub(dlt[:], rot_sl, u_sl)
                nc.vector.tensor_mul(
                    tb[:], dlt[:],
                    ap(g16[:], 128 * CT * c, [[128, CT], [16, 8], [1, 16]]))
                nc.vector.tensor_add(sb[:], u_sl, tb[:])
                nc.scalar.activation(eh[:], sb[:], AF.Exp,
                                     bias=nb100[:], scale=100.0)
                return eh

            def pc_s2(c, eh):
                ns = pc.tile([P, CT, 8], f32, tag="ns")
                r1 = pc.tile([P, CT, 8], f32, tag="r1")
                A16 = pc.tile([P, CT, 8, 16], f16, tag="A16")
                e2 = pc.tile([P, CT, 8, 16], f32, tag="e2")
                nc.vector.tensor_reduce(
                    ns[:], eh[:].rearrange("p a b e -> p (a b) e"),
                    axis=AX.X, op=OP.add)
                nc.vector.reciprocal(r1[:], ns[:])
                nc.gpsimd.tensor_mul(
                    A16[:], eh[:],
                    ap(r1[:], 0, [[8, CT], [1, 8], [0, 16]]))
                nc.scalar.activation(e2[:], A16[:], AF.Exp,
                                     bias=nb100[:], scale=100.0)
                return e2

            def pc_s3(c, e2, tail=False):
                s2 = pc.tile([P, CT, 8], f32, tag="s2")
                r2 = pc.tile([P, CT, 8], f32, tag="r2")
                e2h = pc.tile([P, CT, 4, 16], f16, tag="e2h")
                e2l = pc.tile([P, CT, 4, 16], f16, tag="e2l")
                nc.vector.tensor_reduce(
                    s2[:], e2[:].rearrange("p a b e -> p (a b) e"),
                    axis=AX.X, op=OP.add)
                nc.vector.reciprocal(r2[:], s2[:])
                nc.vector.tensor_mul(
                    e2h[:],
                    ap(e2[:], 16, [[128, CT], [32, 4], [1, 16]]),
                    ap(r2[:], 1, [[8, CT], [2, 4], [0, 16]]))
                eng_l = nc.vector if tail else nc.gpsimd
                eng_l.tensor_mul(
                    e2l[:],
                    ap(e2[:], 0, [[128, CT], [32, 4], [1, 16]]),
                    ap(r2[:], 0, [[8, CT], [2, 4], [0, 16]]))
                return e2h, e2l

            def pc_s4(c, e2h, e2l, tail=False):
                for gsub in range(CT // 4):
                    rep = prep.tile([P, 4, 4, 16, 16], f16, tag="rep")
                    nc.scalar.copy(
                        rep[:],
                        ap(e2h[:], 256 * gsub,
                           [[64, 4], [16, 4], [1, 16], [0, 16]]))
                    for t in range(4):
                        i = CT * c + 4 * gsub + t
                        o_t = pout.tile([P, 4, 16, 16], f16, tag="o_t")
                        nc.vector.tensor_mul(
                            o_t[:],
                            ap(rep[:], 1024 * t, [[256, 4], [16, 16], [1, 16]]),
                            ap(e2l[:], 64 * (4 * gsub + t),
                               [[16, 4], [0, 16], [1, 16]]))
                        nc.sync.dma_start(
                            o_v[i], o_t[:].rearrange("p a b c -> p (a b c)"))

            # ================= emission schedule =================
            # software-pipelined phase A; carry chain per 8-tile quarter;
            # phase-C chunk q stages spread over tiles 8q+9 .. 8q+15 so only
            # the last quarter's chunk runs after phase A ends.
            pending = {}  # c -> (stage, payload)

            def advance_chunk(c, tail=False):
                st, payload = pending.get(c, (0, None))
                if st == 0:
                    pending[c] = (1, pc_s1(c, tail))
                elif st == 1:
                    pending[c] = (2, pc_s2(c, payload))
                elif st == 2:
                    pending[c] = (3, pc_s3(c, payload, tail))
                elif st == 3:
                    pc_s4(c, *payload, tail=tail)
                    pending[c] = (4, None)

            sched = {}
            for q in range(3):
                for k in range(4):
                    sched.setdefault(4 * q + 5 + k, []).append(q)

            NU = NT // 2
            from collections import deque
            prevs = deque()
            for i in range(NU):
                prevs.append((i, phase_a1(i)))
                if i >= 1:
                    j, payload = prevs.popleft()
                    phase_a2(j, *payload)
                if i >= 4 and i % 4 == 0:
                    chain_q(i // 4 - 1)
                for c in sched.get(i, []):
                    advance_chunk(c)
            while prevs:
                j, payload = prevs.popleft()
                phase_a2(j, *payload)
            chain_q(3)
            for c in range(3):  # flush any unfinished in-loop chunks
                while pending.get(c, (0, None))[0] < 4:
                    advance_chunk(c)
            for _ in range(4):
                advance_chunk(3, tail=True)

    nc.compile()
    return nc


def _get_nc():
    global _BUILT
    if _BUILT is None:
        _BUILT = _build()
    return _BUILT


def kernel(a, b, add_table=None, carry_table=None, b2n=None, n2b=None, **_kw):
    from concourse.bass_utils import run_bass_kernel_spmd

    a = np.ascontiguousarray(np.asarray(a, dtype=np.float32))
    b = np.ascontiguousarray(np.asarray(b, dtype=np.float32))
    nc = _get_nc()
    in_maps = [
        {"a": a[i * B_CORE:(i + 1) * B_CORE], "b": b[i * B_CORE:(i + 1) * B_CORE]}
        for i in range(N_CORES)
    ]
    res = run_bass_kernel_spmd(nc, in_maps, core_ids=list(range(N_CORES)))
    out = np.concatenate([r["out"] for r in res.results], axis=0)
    return out.astype(np.float32)


# revision 33
# speedup vs baseline: 2.3143x; 1.0084x over previous
"""Trainium2 Bass kernel for nn_MoEALU (soft ripple-carry byte adder), v3.

Restructured math (validated in sim.py against the jax reference):
  - nibble sums: segmented sums of the 256-wide byte distribution per pos.
  - softmax1 kept UNNORMALIZED (te = exp(100(c - max))); the normalizer
    kappa_s = 1/(sum te_a * sum te_b) = 1/sum_m u_raw[m] is folded into the
    17 conv outputs afterwards.
  - cyclic conv u[m] = sum_i xa_i xb_{(m-i)%16} via a doubled-xb buffer
    (stride [+1 m, -1 i] AP); z15 == u[15].
  - Z1 = 1 - sum_i xa_i p[15-i] where p = prefix sums of xb (one scan).
  - carry chain: softmax over 2 == sigmoid => gamma' = sig(100(d + e*gamma)),
    d = Z1-Z0 = 1-2W, e = 2*z15. At temp 100 gamma saturates to {0,1}, so the
    recurrence linearizes EXACTLY (validated): gamma' = v0 + (v1-v0)*gamma
    with v0 = sig(100 d), v1 = sig(100(d+e)) => one tensor_tensor_scan with
    per-tile reset via zeroed b-coefficient.
  - s-logits: s = u + (rot1(u) - u)*gamma_in.
  - output softmax factorizes: softmax_256(100(sh_i + sl_j)) =
    softmax_16(100 sh) (x) softmax_16(100 sl): two 16-wide softmaxes and an
    outer product per byte; chain softmaxes use the fixed offset exp(100v-100).
fp16: te / conv products / u storage / A / final outer + DMA-out. exp outputs
eh/e2 stay fp32 (fp16 underflows for near-flat dists); per-side r2 folds
(r2h*r2l overflows fp32).

Schedule: phase A software-pipelined per tile; carry chain + phase C run per
16-tile half, with phase C emitted in 8-tile chunks x 4 stages interleaved
into the phase-A stream so DVE never stalls on ACT exps.

Sharding: pure data parallel over batch, 8 cores x 4096 rows.
"""

import numpy as np

B_FULL = 32768
N_CORES = 8
B_CORE = B_FULL // N_CORES  # 4096
P = 128
NT = B_CORE // P  # 32 tiles
HT = NT // 2  # tiles per half
CT = 8  # tiles per phase-C chunk

_BUILT = None


def _build():
    import concourse.bass as bass
    import concourse.bacc as bacc
    import concourse.mybir as mybir
    import concourse.tile as tile

    f32 = mybir.dt.float32
    f16 = mybir.dt.float16
    AF = mybir.ActivationFunctionType
    AX = mybir.AxisListType
    OP = mybir.AluOpType

    nc = bacc.Bacc("TRN2", target_bir_lowering=False, debug=False)
    a_d = nc.dram_tensor("a", [B_CORE, 4, 256], f32, kind="ExternalInput")
    b_d = nc.dram_tensor("b", [B_CORE, 4, 256], f32, kind="ExternalInput")
    out_d = nc.dram_tensor("out", [B_CORE, 4, 256], f16, kind="ExternalOutput")

    def ap(base_ap, off, dims):
        part = base_ap.ap[0]
        return bass.AP(base_ap.tensor, base_ap.offset + off,
                       [list(part)] + [list(d) for d in dims])

    with tile.TileContext(nc) as tc:
        with (
            tc.tile_pool(name="persist", bufs=1) as pp,
            tc.tile_pool(name="pin", bufs=2) as pin,
            tc.tile_pool(name="pa", bufs=2) as pa,
            tc.tile_pool(name="pa1", bufs=2) as pa1,
            tc.tile_pool(name="pc", bufs=2) as pc,
            tc.tile_pool(name="prep", bufs=2) as prep,
            tc.tile_pool(name="pout", bufs=3) as pout,
        ):
            # ---------------- persistent tensors ----------------
            u_all = pp.tile([P, NT, 8, 18], f16, tag="u_all")
            d_all = pp.tile([P, NT, 8], f32, tag="d_all")
            e_all = pp.tile([P, NT, 8], f32, tag="e_all")
            dpe = pp.tile([P, NT, 8], f32, tag="dpe")
            v0 = pp.tile([P, NT, 8], f32, tag="v0")
            v1 = pp.tile([P, NT, 8], f32, tag="v1")
            bco = pp.tile([P, NT, 8], f32, tag="bco")
            gg = pp.tile([P, 257], f32, tag="gg")
            g16 = pp.tile([P, NT, 8, 16], f16, tag="g16")
            nb100 = pp.tile([P, 1], f32, tag="nb100")
            nc.gpsimd.memset(nb100[:], -100.0)
            nc.gpsimd.memset(ap(bco[:], 0, [[8, NT]]), 0.0)

            a_v = a_d.ap().rearrange("(n p) f g -> n p (f g)", p=P)
            b_v = b_d.ap().rearrange("(n p) f g -> n p (f g)", p=P)
            o_v = out_d.ap().rearrange("(n p) f g -> n p (f g)", p=P)

            # ----- phase A stage 1 (2-tile super-tile): load+sums+max ----
            # ab layout [tensor, t, 1024] so (tensor,t) merges to stride 1024
            def phase_a1(u):
                ab = pin.tile([P, 2, 2, 1024], f32, tag="ab")
                order = ((0, 0), (0, 1), (1, 0), (1, 1)) if u == 0 else \
                    ((0, 0), (1, 0), (0, 1), (1, 1))
                for T2, t in order:
                    nc.sync.dma_start(
                        ap(ab[:], 2048 * T2 + 1024 * t, [[1, 1024]]),
                        (a_v if T2 == 0 else b_v)[2 * u + t])
                # c_all [tensor, t, 8 stages, 16]
                c_all = pa1.tile([P, 512], f32, tag="c_all")
                lo1 = pa.tile([P, 4, 4, 16, 8], f32, tag="lo1")
                if u == 0:
                    # split per tensor-half so compute starts after 2 DMAs
                    for T2 in range(2):
                        nc.vector.tensor_reduce(
                            ap(c_all[:], 256 * T2 + 16,
                               [[128, 2], [32, 4], [1, 16]]),
                            ap(ab[:], 2048 * T2,
                               [[1024, 2], [256, 4], [16, 16], [1, 16]]),
                            axis=AX.X, op=OP.add)
                        nc.gpsimd.tensor_add(
                            ap(lo1[:], 1024 * T2,
                               [[512, 2], [128, 4], [8, 16], [1, 8]]),
                            ap(ab[:], 2048 * T2,
                               [[1024, 2], [256, 4], [1, 16], [16, 8]]),
                            ap(ab[:], 2048 * T2 + 128,
                               [[1024, 2], [256, 4], [1, 16], [16, 8]]))
                else:
                    nc.vector.tensor_reduce(
                        ap(c_all[:], 16, [[128, 4], [32, 4], [1, 16]]),
                        ap(ab[:], 0, [[1024, 4], [256, 4], [16, 16], [1, 16]]),
                        axis=AX.X, op=OP.add)
                    nc.gpsimd.tensor_add(
                        lo1[:],
                        ap(ab[:], 0, [[1024, 4], [256, 4], [1, 16], [16, 8]]),
                        ap(ab[:], 128, [[1024, 4], [256, 4], [1, 16], [16, 8]]))
                lo2 = pa.tile([P, 4, 4, 16, 4], f32, tag="lo2")
                nc.gpsimd.tensor_add(
                    lo2[:],
                    ap(lo1[:], 0, [[512, 4], [128, 4], [8, 16], [1, 4]]),
                    ap(lo1[:], 4, [[512, 4], [128, 4], [8, 16], [1, 4]]))
                lo3 = pa.tile([P, 4, 4, 16, 2], f32, tag="lo3")
                nc.gpsimd.tensor_add(
                    lo3[:],
                    ap(lo2[:], 0, [[256, 4], [64, 4], [4, 16], [1, 2]]),
                    ap(lo2[:], 2, [[256, 4], [64, 4], [4, 16], [1, 2]]))
                nc.gpsimd.tensor_add(
                    ap(c_all[:], 0, [[128, 4], [32, 4], [1, 16]]),
                    ap(lo3[:], 0, [[128, 4], [32, 4], [2, 16]]),
                    ap(lo3[:], 1, [[128, 4], [32, 4], [2, 16]]))
                m16 = pa1.tile([P, 32], f32, tag="m16")
                nc.vector.tensor_reduce(
                    m16[:], c_all[:].rearrange("p (g e) -> p g e", g=32),
                    axis=AX.X, op=OP.max)
                return c_all, m16

            # ----- phase A stage 2 (2-tile super-tile) -----
            def phase_a2(u, c_all, m16):
                ts = pa.tile([P, 512], f32, tag="ts")
                nc.gpsimd.tensor_sub(
                    ts[:].rearrange("p (g e) -> p g e", g=32),
                    c_all[:].rearrange("p (g e) -> p g e", g=32),
                    ap(m16[:], 0, [[1, 32], [0, 16]]))
                # T [tensor, t, s, 16]: b-half contiguous at 256 for the scan
                T = pa.tile([P, 2, 2, 8, 16], f16, tag="T")
                nc.scalar.activation(
                    T[:].rearrange("p a b c d -> p (a b c d)"),
                    ts[:], AF.Exp, scale=100.0)
                xbd = pa.tile([P, 2, 8, 32], f16, tag="xbd")
                nc.scalar.copy(
                    ap(xbd[:], 0, [[256, 2], [32, 8], [1, 16]]),
                    ap(T[:], 256, [[128, 2], [16, 8], [1, 16]]))
                nc.scalar.copy(
                    ap(xbd[:], 16, [[256, 2], [32, 8], [1, 16]]),
                    ap(T[:], 256, [[128, 2], [16, 8], [1, 16]]))
                S = pa.tile([P, 260], f32, tag="S")
                nc.gpsimd.memset(ap(S[:], 0, [[1, 1]]), 0.0)
                nc.vector.tensor_tensor_scan(
                    ap(S[:], 1, [[1, 256]]),
                    ap(T[:], 256, [[1, 256]]),
                    ap(T[:], 256, [[1, 256]]),
                    0.0, OP.add, OP.bypass)
                p16 = pa.tile([P, 2, 8, 16], f16, tag="p16")
                nc.vector.tensor_sub(
                    p16[:],
                    ap(S[:], 1, [[16, 16], [1, 16]]),
                    ap(S[:], 0, [[16, 16], [0, 16]]))
                q = pa.tile([P, 2, 8, 17, 16], f16, tag="q")
                nc.vector.tensor_mul(
                    ap(q[:], 0, [[2176, 2], [272, 8], [16, 16], [1, 16]]),
                    ap(T[:], 0, [[128, 2], [16, 8], [0, 16], [1, 16]]),
                    ap(xbd[:], 16, [[256, 2], [32, 8], [1, 16], [-1, 16]]))
                nc.vector.tensor_mul(
                    ap(q[:], 256, [[2176, 2], [272, 8], [1, 16]]),
                    ap(T[:], 0, [[128, 2], [16, 8], [1, 16]]),
                    ap(p16[:], 15, [[128, 2], [16, 8], [-1, 16]]))
                qt1 = pa.tile([P, 2, 8, 17, 8], f16, tag="qt1")
                nc.vector.tensor_add(
                    qt1[:],
                    ap(q[:], 0, [[2176, 2], [272, 8], [16, 17], [1, 8]]),
                    ap(q[:], 8, [[2176, 2], [272, 8], [16, 17], [1, 8]]))
                qt2 = pa.tile([P, 2, 8, 17, 4], f16, tag="qt2")
                nc.vector.tensor_add(
                    qt2[:],
                    ap(qt1[:], 0, [[1088, 2], [136, 8], [8, 17], [1, 4]]),
                    ap(qt1[:], 4, [[1088, 2], [136, 8], [8, 17], [1, 4]]))
                qt3 = pa.tile([P, 2, 8, 17, 2], f16, tag="qt3")
                nc.vector.tensor_add(
                    qt3[:],
                    ap(qt2[:], 0, [[544, 2], [68, 8], [4, 17], [1, 2]]),
                    ap(qt2[:], 2, [[544, 2], [68, 8], [4, 17], [1, 2]]))
                qr = pa.tile([P, 2, 8, 17], f16, tag="qr")
                nc.vector.tensor_add(
                    qr[:],
                    ap(qt3[:], 0, [[272, 2], [34, 8], [2, 17]]),
                    ap(qt3[:], 1, [[272, 2], [34, 8], [2, 17]]))
                Su = pa.tile([P, 16], f32, tag="Su")
                nc.vector.tensor_reduce(
                    Su[:], ap(qr[:], 0, [[17, 16], [1, 16]]),
                    axis=AX.X, op=OP.add)
                rk = pa.tile([P, 16], f32, tag="rk")
                nc.vector.reciprocal(rk[:], Su[:])
                nc.gpsimd.tensor_mul(
                    ap(u_all[:], 288 * u + 1, [[144, 2], [18, 8], [1, 16]]),
                    ap(qr[:], 0, [[136, 2], [17, 8], [1, 16]]),
                    ap(rk[:], 0, [[8, 2], [1, 8], [0, 16]]))
                nc.gpsimd.tensor_mul(
                    ap(u_all[:], 288 * u, [[144, 2], [18, 8]]),
                    ap(qr[:], 15, [[136, 2], [17, 8]]),
                    ap(rk[:], 0, [[8, 2], [1, 8]]))
                t8 = pa.tile([P, 16], f32, tag="t8")
                nc.gpsimd.tensor_mul(
                    t8[:], ap(qr[:], 16, [[136, 2], [17, 8]]),
                    ap(rk[:], 0, [[8, 2], [1, 8]]))
                nc.vector.tensor_scalar(
                    ap(d_all[:], 16 * u, [[1, 16]]), t8[:],
                    -2.0, 1.0, op0=OP.mult, op1=OP.add)
                nc.vector.tensor_scalar_mul(
                    ap(e_all[:], 16 * u, [[1, 16]]),
                    ap(u_all[:], 288 * u, [[144, 2], [18, 8]]), 2.0)

            # --------- carry chain for an 8-tile quarter ---------
            def chain_q(h):
                o = 8 * CT * h  # 64 per quarter
                dsl = ap(d_all[:], o, [[1, 8 * CT]])
                esl = ap(e_all[:], o, [[1, 8 * CT]])
                psl = ap(dpe[:], o, [[1, 8 * CT]])
                v0s = ap(v0[:], o, [[1, 8 * CT]])
                v1s = ap(v1[:], o, [[1, 8 * CT]])
                nc.gpsimd.tensor_add(psl, dsl, esl)
                nc.scalar.activation(v0s, dsl, AF.Sigmoid, scale=100.0)
                nc.scalar.activation(v1s, psl, AF.Sigmoid, scale=100.0)
                nc.gpsimd.tensor_sub(
                    ap(bco[:], o + 1, [[8, CT], [1, 7]]),
                    ap(v1[:], o + 1, [[8, CT], [1, 7]]),
                    ap(v0[:], o + 1, [[8, CT], [1, 7]]))
                nc.vector.tensor_tensor_scan(
                    ap(gg[:], o + 1, [[1, 8 * CT]]),
                    ap(bco[:], o, [[1, 8 * CT]]),
                    ap(v0[:], o, [[1, 8 * CT]]),
                    0.0, OP.mult, OP.add)
                # gg[o+8t] slots hold only discarded stage-7 carry-outs
                nc.gpsimd.memset(ap(gg[:], o, [[8, CT]]), 0.0)
                nc.scalar.copy(
                    ap(g16[:], 128 * CT * h, [[128, CT], [16, 8], [1, 16]]),
                    ap(gg[:], o, [[8, CT], [1, 8], [0, 16]]))

            # --------- phase C chunk stages (CT=8 tiles each) ---------
            def pc_s1(c, tail=False):
                o = 144 * CT * c
                u_sl = ap(u_all[:], o + 1, [[144, CT], [18, 8], [1, 16]])
                rot_sl = ap(u_all[:], o, [[144, CT], [18, 8], [1, 16]])
                dlt = pc.tile([P, CT, 8, 16], f16, tag="dlt")
                tb = pc.tile([P, CT, 8, 16], f16, tag="tb")
                sb = pc.tile([P, CT, 8, 16], f16, tag="sb")
                eh = pc.tile([P, CT, 8, 16], f32, tag="eh")
                nc.vector.tensor_sub(dlt[:], rot_sl, u_sl)
                nc.vector.tensor_mul(
                    tb[:], dlt[:],
                    ap(g16[:], 128 * CT * c, [[128, CT], [16, 8], [1, 16]]))
                nc.vector.tensor_add(sb[:], u_sl, tb[:])
                nc.scalar.activation(eh[:], sb[:], AF.Exp,
                                     bias=nb100[:], scale=100.0)
                return eh

            def pc_s2(c, eh):
                ns = pc.tile([P, CT, 8], f32, tag="ns")
                r1 = pc.tile([P, CT, 8], f32, tag="r1")
                A16 = pc.tile([P, CT, 8, 16], f16, tag="A16")
                e2 = pc.tile([P, CT, 8, 16], f32, tag="e2")
                nc.vector.tensor_reduce(
                    ns[:], eh[:].rearrange("p a b e -> p (a b) e"),
                    axis=AX.X, op=OP.add)
                nc.vector.reciprocal(r1[:], ns[:])
                nc.gpsimd.tensor_mul(
                    A16[:], eh[:],
                    ap(r1[:], 0, [[8, CT], [1, 8], [0, 16]]))
                nc.scalar.activation(e2[:], A16[:], AF.Exp,
                                     bias=nb100[:], scale=100.0)
                return e2

            def pc_s3(c, e2, tail=False):
                s2 = pc.tile([P, CT, 8], f32, tag="s2")
                r2 = pc.tile([P, CT, 8], f32, tag="r2")
                e2h = pc.tile([P, CT, 4, 16], f16, tag="e2h")
                e2l = pc.tile([P, CT, 4, 16], f16, tag="e2l")
                nc.vector.tensor_reduce(
                    s2[:], e2[:].rearrange("p a b e -> p (a b) e"),
                    axis=AX.X, op=OP.add)
                nc.vector.reciprocal(r2[:], s2[:])
                nc.vector.tensor_mul(
                    e2h[:],
                    ap(e2[:], 16, [[128, CT], [32, 4], [1, 16]]),
                    ap(r2[:], 1, [[8, CT], [2, 4], [0, 16]]))
                eng_l = nc.vector if tail else nc.gpsimd
                eng_l.tensor_mul(
                    e2l[:],
                    ap(e2[:], 0, [[128, CT], [32, 4], [1, 16]]),
                    ap(r2[:], 0, [[8, CT], [2, 4], [0, 16]]))
                return e2h, e2l

            def pc_s4(c, e2h, e2l, tail=False):
                for gsub in range(CT // 4):
                    rep = prep.tile([P, 4, 4, 16, 16], f16, tag="rep")
                    nc.scalar.copy(
                        rep[:],
                        ap(e2h[:], 256 * gsub,
                           [[64, 4], [16, 4], [1, 16], [0, 16]]))
                    for t in range(4):
                        i = CT * c + 4 * gsub + t
                        o_t = pout.tile([P, 4, 16, 16], f16, tag="o_t")
                        nc.vector.tensor_mul(
                            o_t[:],
                            ap(rep[:], 1024 * t, [[256, 4], [16, 16], [1, 16]]),
                            ap(e2l[:], 64 * (4 * gsub + t),
                               [[16, 4], [0, 16], [1, 16]]))
                        nc.sync.dma_start(
                            o_v[i], o_t[:].rearrange("p a b c -> p (a b c)"))

            # ================= emission schedule =================
            # software-pipelined phase A; carry chain per 8-tile quarter;
            # phase-C chunk q stages spread over tiles 8q+9 .. 8q+15 so only
            # the last quarter's chunk runs after phase A ends.
            pending = {}  # c -> (stage, payload)

            def advance_chunk(c, tail=False):
                st, payload = pending.get(c, (0, None))
                if st == 0:
                    pending[c] = (1, pc_s1(c, tail))
                elif st == 1:
                    pending[c] = (2, pc_s2(c, payload))
                elif st == 2:
                    pending[c] = (3, pc_s3(c, payload, tail))
                elif st == 3:
                    pc_s4(c, *payload, tail=tail)
                    pending[c] = (4, None)

            sched = {}
            for q in range(3):
                for k in range(4):
                    sched.setdefault(4 * q + 5 + k, []).append(q)

            NU = NT // 2
            from collections import deque
            prevs = deque()
            for i in range(NU):
                prevs.append((i, phase_a1(i)))
                if i >= 1:
                    j, payload = prevs.popleft()
                    phase_a2(j, *payload)
                if i >= 4 and i % 4 == 0:
                    chain_q(i // 4 - 1)
                for c in sched.get(i, []):
                    advance_chunk(c)
            while prevs:
                j, payload = prevs.popleft()
                phase_a2(j, *payload)
            chain_q(3)
            for c in range(3):  # flush any unfinished in-loop chunks
                while pending.get(c, (0, None))[0] < 4:
                    advance_chunk(c)
            for _ in range(4):
                advance_chunk(3, tail=True)

    nc.compile()
    return nc


def _get_nc():
    global _BUILT
    if _BUILT is None:
        _BUILT = _build()
    return _BUILT


def kernel(a, b, add_table=None, carry_table=None, b2n=None, n2b=None, **_kw):
    from concourse.bass_utils import run_bass_kernel_spmd

    a = np.ascontiguousarray(np.asarray(a, dtype=np.float32))
    b = np.ascontiguousarray(np.asarray(b, dtype=np.float32))
    nc = _get_nc()
    in_maps = [
        {"a": a[i * B_CORE:(i + 1) * B_CORE], "b": b[i * B_CORE:(i + 1) * B_CORE]}
        for i in range(N_CORES)
    ]
    res = run_bass_kernel_spmd(nc, in_maps, core_ids=list(range(N_CORES)))
    out = np.concatenate([r["out"] for r in res.results], axis=0)
    return out.astype(np.float32)


# revision 35
# speedup vs baseline: 2.3269x; 1.0055x over previous
"""Trainium2 Bass kernel for nn_MoEALU (soft ripple-carry byte adder), v3.

Restructured math (validated in sim.py against the jax reference):
  - nibble sums: segmented sums of the 256-wide byte distribution per pos.
  - softmax1 kept UNNORMALIZED (te = exp(100(c - max))); the normalizer
    kappa_s = 1/(sum te_a * sum te_b) = 1/sum_m u_raw[m] is folded into the
    17 conv outputs afterwards.
  - cyclic conv u[m] = sum_i xa_i xb_{(m-i)%16} via a doubled-xb buffer
    (stride [+1 m, -1 i] AP); z15 == u[15].
  - Z1 = 1 - sum_i xa_i p[15-i] where p = prefix sums of xb (one scan).
  - carry chain: softmax over 2 == sigmoid => gamma' = sig(100(d + e*gamma)),
    d = Z1-Z0 = 1-2W, e = 2*z15. At temp 100 gamma saturates to {0,1}, so the
    recurrence linearizes EXACTLY (validated): gamma' = v0 + (v1-v0)*gamma
    with v0 = sig(100 d), v1 = sig(100(d+e)) => one tensor_tensor_scan with
    per-tile reset via zeroed b-coefficient.
  - s-logits: s = u + (rot1(u) - u)*gamma_in.
  - output softmax factorizes: softmax_256(100(sh_i + sl_j)) =
    softmax_16(100 sh) (x) softmax_16(100 sl): two 16-wide softmaxes and an
    outer product per byte; chain softmaxes use the fixed offset exp(100v-100).
fp16: te / conv products / u storage / A / final outer + DMA-out. exp outputs
eh/e2 stay fp32 (fp16 underflows for near-flat dists); per-side r2 folds
(r2h*r2l overflows fp32).

Schedule: phase A software-pipelined per tile; carry chain + phase C run per
16-tile half, with phase C emitted in 8-tile chunks x 4 stages interleaved
into the phase-A stream so DVE never stalls on ACT exps.

Sharding: pure data parallel over batch, 8 cores x 4096 rows.
"""

import numpy as np

B_FULL = 32768
N_CORES = 8
B_CORE = B_FULL // N_CORES  # 4096
P = 128
NT = B_CORE // P  # 32 tiles
HT = NT // 2  # tiles per half
CT = 8  # tiles per phase-C chunk

_BUILT = None


def _build():
    import concourse.bass as bass
    import concourse.bacc as bacc
    import concourse.mybir as mybir
    import concourse.tile as tile

    f32 = mybir.dt.float32
    f16 = mybir.dt.float16
    AF = mybir.ActivationFunctionType
    AX = mybir.AxisListType
    OP = mybir.AluOpType

    nc = bacc.Bacc("TRN2", target_bir_lowering=False, debug=False)
    a_d = nc.dram_tensor("a", [B_CORE, 4, 256], f32, kind="ExternalInput")
    b_d = nc.dram_tensor("b", [B_CORE, 4, 256], f32, kind="ExternalInput")
    out_d = nc.dram_tensor("out", [B_CORE, 4, 256], f16, kind="ExternalOutput")

    def ap(base_ap, off, dims):
        part = base_ap.ap[0]
        return bass.AP(base_ap.tensor, base_ap.offset + off,
                       [list(part)] + [list(d) for d in dims])

    with tile.TileContext(nc) as tc:
        with (
            tc.tile_pool(name="persist", bufs=1) as pp,
            tc.tile_pool(name="pin", bufs=2) as pin,
            tc.tile_pool(name="pa", bufs=2) as pa,
            tc.tile_pool(name="pa1", bufs=2) as pa1,
            tc.tile_pool(name="pc", bufs=2) as pc,
            tc.tile_pool(name="prep", bufs=2) as prep,
            tc.tile_pool(name="pout", bufs=3) as pout,
        ):
            # ---------------- persistent tensors ----------------
            u_all = pp.tile([P, NT, 8, 18], f16, tag="u_all")
            d_all = pp.tile([P, NT, 8], f32, tag="d_all")
            e_all = pp.tile([P, NT, 8], f32, tag="e_all")
            dpe = pp.tile([P, NT, 8], f32, tag="dpe")
            v0 = pp.tile([P, NT, 8], f32, tag="v0")
            v1 = pp.tile([P, NT, 8], f32, tag="v1")
            bco = pp.tile([P, NT, 8], f32, tag="bco")
            gg = pp.tile([P, 257], f32, tag="gg")
            g16 = pp.tile([P, NT, 8, 16], f16, tag="g16")
            nb100 = pp.tile([P, 1], f32, tag="nb100")
            nc.gpsimd.memset(nb100[:], -100.0)
            nc.gpsimd.memset(ap(bco[:], 0, [[8, NT]]), 0.0)

            a_v = a_d.ap().rearrange("(n p) f g -> n p (f g)", p=P)
            b_v = b_d.ap().rearrange("(n p) f g -> n p (f g)", p=P)
            o_v = out_d.ap().rearrange("(n p) f g -> n p (f g)", p=P)

            # ----- phase A stage 1 (2-tile super-tile): load+sums+max ----
            # ab layout [tensor, t, 1024] so (tensor,t) merges to stride 1024
            def phase_a1(u):
                ab = pin.tile([P, 2, 2, 1024], f32, tag="ab")
                order = ((0, 0), (0, 1), (1, 0), (1, 1)) if u == 0 else \
                    ((0, 0), (1, 0), (0, 1), (1, 1))
                for T2, t in order:
                    nc.sync.dma_start(
                        ap(ab[:], 2048 * T2 + 1024 * t, [[1, 1024]]),
                        (a_v if T2 == 0 else b_v)[2 * u + t])
                # c_all [tensor, t, 8 stages, 16]
                c_all = pa1.tile([P, 512], f32, tag="c_all")
                lo1 = pa.tile([P, 4, 4, 16, 8], f32, tag="lo1")
                if u == 0:
                    # split per tensor-half so compute starts after 2 DMAs
                    for T2 in range(2):
                        nc.vector.tensor_reduce(
                            ap(c_all[:], 256 * T2 + 16,
                               [[128, 2], [32, 4], [1, 16]]),
                            ap(ab[:], 2048 * T2,
                               [[1024, 2], [256, 4], [16, 16], [1, 16]]),
                            axis=AX.X, op=OP.add)
                        nc.gpsimd.tensor_add(
                            ap(lo1[:], 1024 * T2,
                               [[512, 2], [128, 4], [8, 16], [1, 8]]),
                            ap(ab[:], 2048 * T2,
                               [[1024, 2], [256, 4], [1, 16], [16, 8]]),
                            ap(ab[:], 2048 * T2 + 128,
                               [[1024, 2], [256, 4], [1, 16], [16, 8]]))
                else:
                    nc.vector.tensor_reduce(
                        ap(c_all[:], 16, [[128, 4], [32, 4], [1, 16]]),
                        ap(ab[:], 0, [[1024, 4], [256, 4], [16, 16], [1, 16]]),
                        axis=AX.X, op=OP.add)
                    nc.gpsimd.tensor_add(
                        lo1[:],
                        ap(ab[:], 0, [[1024, 4], [256, 4], [1, 16], [16, 8]]),
                        ap(ab[:], 128, [[1024, 4], [256, 4], [1, 16], [16, 8]]))
                lo2 = pa.tile([P, 4, 4, 16, 4], f32, tag="lo2")
                nc.gpsimd.tensor_add(
                    lo2[:],
                    ap(lo1[:], 0, [[512, 4], [128, 4], [8, 16], [1, 4]]),
                    ap(lo1[:], 4, [[512, 4], [128, 4], [8, 16], [1, 4]]))
                lo3 = pa.tile([P, 4, 4, 16, 2], f32, tag="lo3")
                nc.gpsimd.tensor_add(
                    lo3[:],
                    ap(lo2[:], 0, [[256, 4], [64, 4], [4, 16], [1, 2]]),
                    ap(lo2[:], 2, [[256, 4], [64, 4], [4, 16], [1, 2]]))
                nc.gpsimd.tensor_add(
                    ap(c_all[:], 0, [[128, 4], [32, 4], [1, 16]]),
                    ap(lo3[:], 0, [[128, 4], [32, 4], [2, 16]]),
                    ap(lo3[:], 1, [[128, 4], [32, 4], [2, 16]]))
                m16 = pa1.tile([P, 32], f32, tag="m16")
                nc.vector.tensor_reduce(
                    m16[:], c_all[:].rearrange("p (g e) -> p g e", g=32),
                    axis=AX.X, op=OP.max)
                return c_all, m16

            # ----- phase A stage 2 (2-tile super-tile) -----
            def phase_a2(u, c_all, m16):
                ts = pa.tile([P, 512], f32, tag="ts")
                nc.gpsimd.tensor_sub(
                    ts[:].rearrange("p (g e) -> p g e", g=32),
                    c_all[:].rearrange("p (g e) -> p g e", g=32),
                    ap(m16[:], 0, [[1, 32], [0, 16]]))
                # T [tensor, t, s, 16]: b-half contiguous at 256 for the scan
                T = pa.tile([P, 2, 2, 8, 16], f16, tag="T")
                nc.scalar.activation(
                    T[:].rearrange("p a b c d -> p (a b c d)"),
                    ts[:], AF.Exp, scale=100.0)
                xbd = pa.tile([P, 2, 8, 32], f16, tag="xbd")
                nc.scalar.copy(
                    ap(xbd[:], 0, [[256, 2], [32, 8], [1, 16]]),
                    ap(T[:], 256, [[128, 2], [16, 8], [1, 16]]))
                nc.scalar.copy(
                    ap(xbd[:], 16, [[256, 2], [32, 8], [1, 16]]),
                    ap(T[:], 256, [[128, 2], [16, 8], [1, 16]]))
                S = pa.tile([P, 260], f32, tag="S")
                nc.gpsimd.memset(ap(S[:], 0, [[1, 1]]), 0.0)
                nc.vector.tensor_tensor_scan(
                    ap(S[:], 1, [[1, 256]]),
                    ap(T[:], 256, [[1, 256]]),
                    ap(T[:], 256, [[1, 256]]),
                    0.0, OP.add, OP.bypass)
                p16 = pa.tile([P, 2, 8, 16], f16, tag="p16")
                nc.vector.tensor_sub(
                    p16[:],
                    ap(S[:], 1, [[16, 16], [1, 16]]),
                    ap(S[:], 0, [[16, 16], [0, 16]]))
                q = pa.tile([P, 2, 8, 17, 16], f16, tag="q")
                nc.vector.tensor_mul(
                    ap(q[:], 0, [[2176, 2], [272, 8], [16, 16], [1, 16]]),
                    ap(T[:], 0, [[128, 2], [16, 8], [0, 16], [1, 16]]),
                    ap(xbd[:], 16, [[256, 2], [32, 8], [1, 16], [-1, 16]]))
                nc.vector.tensor_mul(
                    ap(q[:], 256, [[2176, 2], [272, 8], [1, 16]]),
                    ap(T[:], 0, [[128, 2], [16, 8], [1, 16]]),
                    ap(p16[:], 15, [[128, 2], [16, 8], [-1, 16]]))
                qt1 = pa.tile([P, 2, 8, 17, 8], f16, tag="qt1")
                nc.vector.tensor_add(
                    qt1[:],
                    ap(q[:], 0, [[2176, 2], [272, 8], [16, 17], [1, 8]]),
                    ap(q[:], 8, [[2176, 2], [272, 8], [16, 17], [1, 8]]))
                qt2 = pa.tile([P, 2, 8, 17, 4], f16, tag="qt2")
                nc.vector.tensor_add(
                    qt2[:],
                    ap(qt1[:], 0, [[1088, 2], [136, 8], [8, 17], [1, 4]]),
                    ap(qt1[:], 4, [[1088, 2], [136, 8], [8, 17], [1, 4]]))
                qt3 = pa.tile([P, 2, 8, 17, 2], f16, tag="qt3")
                nc.vector.tensor_add(
                    qt3[:],
                    ap(qt2[:], 0, [[544, 2], [68, 8], [4, 17], [1, 2]]),
                    ap(qt2[:], 2, [[544, 2], [68, 8], [4, 17], [1, 2]]))
                qr = pa.tile([P, 2, 8, 17], f16, tag="qr")
                nc.vector.tensor_add(
                    qr[:],
                    ap(qt3[:], 0, [[272, 2], [34, 8], [2, 17]]),
                    ap(qt3[:], 1, [[272, 2], [34, 8], [2, 17]]))
                Su = pa.tile([P, 16], f32, tag="Su")
                nc.vector.tensor_reduce(
                    Su[:], ap(qr[:], 0, [[17, 16], [1, 16]]),
                    axis=AX.X, op=OP.add)
                rk = pa.tile([P, 16], f32, tag="rk")
                nc.vector.reciprocal(rk[:], Su[:])
                nc.gpsimd.tensor_mul(
                    ap(u_all[:], 288 * u + 1, [[144, 2], [18, 8], [1, 16]]),
                    ap(qr[:], 0, [[136, 2], [17, 8], [1, 16]]),
                    ap(rk[:], 0, [[8, 2], [1, 8], [0, 16]]))
                nc.gpsimd.tensor_mul(
                    ap(u_all[:], 288 * u, [[144, 2], [18, 8]]),
                    ap(qr[:], 15, [[136, 2], [17, 8]]),
                    ap(rk[:], 0, [[8, 2], [1, 8]]))
                t8 = pa.tile([P, 16], f32, tag="t8")
                nc.gpsimd.tensor_mul(
                    t8[:], ap(qr[:], 16, [[136, 2], [17, 8]]),
                    ap(rk[:], 0, [[8, 2], [1, 8]]))
                nc.vector.tensor_scalar(
                    ap(d_all[:], 16 * u, [[1, 16]]), t8[:],
                    -2.0, 1.0, op0=OP.mult, op1=OP.add)
                nc.vector.tensor_scalar_mul(
                    ap(e_all[:], 16 * u, [[1, 16]]),
                    ap(u_all[:], 288 * u, [[144, 2], [18, 8]]), 2.0)

            # --------- carry chain for tiles [t0, t0+nt) ---------
            def chain_q(t0, nt=8):
                o = 8 * t0
                dsl = ap(d_all[:], o, [[1, 8 * nt]])
                esl = ap(e_all[:], o, [[1, 8 * nt]])
                psl = ap(dpe[:], o, [[1, 8 * nt]])
                v0s = ap(v0[:], o, [[1, 8 * nt]])
                v1s = ap(v1[:], o, [[1, 8 * nt]])
                nc.gpsimd.tensor_add(psl, dsl, esl)
                nc.scalar.activation(v0s, dsl, AF.Sigmoid, scale=100.0)
                nc.scalar.activation(v1s, psl, AF.Sigmoid, scale=100.0)
                nc.gpsimd.tensor_sub(
                    ap(bco[:], o + 1, [[8, nt], [1, 7]]),
                    ap(v1[:], o + 1, [[8, nt], [1, 7]]),
                    ap(v0[:], o + 1, [[8, nt], [1, 7]]))
                nc.vector.tensor_tensor_scan(
                    ap(gg[:], o + 1, [[1, 8 * nt]]),
                    ap(bco[:], o, [[1, 8 * nt]]),
                    ap(v0[:], o, [[1, 8 * nt]]),
                    0.0, OP.mult, OP.add)
                # gg[o+8t] slots hold only discarded stage-7 carry-outs
                nc.gpsimd.memset(ap(gg[:], o, [[8, nt]]), 0.0)
                nc.scalar.copy(
                    ap(g16[:], 128 * t0, [[128, nt], [16, 8], [1, 16]]),
                    ap(gg[:], o, [[8, nt], [1, 8], [0, 16]]))

            # --------- phase C chunk stages (CT=8 tiles each) ---------
            def pc_s1(t0, ct):
                o = 144 * t0
                u_sl = ap(u_all[:], o + 1, [[144, ct], [18, 8], [1, 16]])
                rot_sl = ap(u_all[:], o, [[144, ct], [18, 8], [1, 16]])
                dlt = pc.tile([P, ct, 8, 16], f16, tag="dlt")
                tb = pc.tile([P, ct, 8, 16], f16, tag="tb")
                sb = pc.tile([P, ct, 8, 16], f16, tag="sb")
                eh = pc.tile([P, ct, 8, 16], f32, tag="eh")
                nc.vector.tensor_sub(dlt[:], rot_sl, u_sl)
                nc.vector.tensor_mul(
                    tb[:], dlt[:],
                    ap(g16[:], 128 * t0, [[128, ct], [16, 8], [1, 16]]))
                nc.vector.tensor_add(sb[:], u_sl, tb[:])
                nc.scalar.activation(eh[:], sb[:], AF.Exp,
                                     bias=nb100[:], scale=100.0)
                return eh

            def pc_s2(t0, ct, eh):
                ns = pc.tile([P, ct, 8], f32, tag="ns")
                r1 = pc.tile([P, ct, 8], f32, tag="r1")
                A16 = pc.tile([P, ct, 8, 16], f16, tag="A16")
                e2 = pc.tile([P, ct, 8, 16], f32, tag="e2")
                nc.vector.tensor_reduce(
                    ns[:], eh[:].rearrange("p a b e -> p (a b) e"),
                    axis=AX.X, op=OP.add)
                nc.vector.reciprocal(r1[:], ns[:])
                nc.gpsimd.tensor_mul(
                    A16[:], eh[:],
                    ap(r1[:], 0, [[8, ct], [1, 8], [0, 16]]))
                nc.scalar.activation(e2[:], A16[:], AF.Exp,
                                     bias=nb100[:], scale=100.0)
                return e2

            def pc_s3(t0, ct, e2, tail=False):
                s2 = pc.tile([P, ct, 8], f32, tag="s2")
                r2 = pc.tile([P, ct, 8], f32, tag="r2")
                e2h = pc.tile([P, ct, 4, 16], f16, tag="e2h")
                e2l = pc.tile([P, ct, 4, 16], f16, tag="e2l")
                nc.vector.tensor_reduce(
                    s2[:], e2[:].rearrange("p a b e -> p (a b) e"),
                    axis=AX.X, op=OP.add)
                nc.vector.reciprocal(r2[:], s2[:])
                nc.vector.tensor_mul(
                    e2h[:],
                    ap(e2[:], 16, [[128, ct], [32, 4], [1, 16]]),
                    ap(r2[:], 1, [[8, ct], [2, 4], [0, 16]]))
                eng_l = nc.vector if tail else nc.gpsimd
                eng_l.tensor_mul(
                    e2l[:],
                    ap(e2[:], 0, [[128, ct], [32, 4], [1, 16]]),
                    ap(r2[:], 0, [[8, ct], [2, 4], [0, 16]]))
                return e2h, e2l

            def pc_s4(t0, ct, e2h, e2l, tail=False):
                for gsub in range(ct // 4):
                    rep = prep.tile([P, 4, 4, 16, 16], f16, tag="rep")
                    nc.scalar.copy(
                        rep[:],
                        ap(e2h[:], 256 * gsub,
                           [[64, 4], [16, 4], [1, 16], [0, 16]]))
                    for t in range(4):
                        i = t0 + 4 * gsub + t
                        o_t = pout.tile([P, 4, 16, 16], f16, tag="o_t")
                        nc.vector.tensor_mul(
                            o_t[:],
                            ap(rep[:], 1024 * t, [[256, 4], [16, 16], [1, 16]]),
                            ap(e2l[:], 64 * (4 * gsub + t),
                               [[16, 4], [0, 16], [1, 16]]))
                        nc.scalar.dma_start(
                            o_v[i], o_t[:].rearrange("p a b c -> p (a b c)"))

            # ================= emission schedule =================
            # software-pipelined phase A; carry chain per 8-tile quarter;
            # phase-C chunk q stages spread over tiles 8q+9 .. 8q+15 so only
            # the last quarter's chunk runs after phase A ends.
            pending = {}  # c -> (stage, payload)

            def advance_chunk(key, tail=False):
                t0, ct = key
                st, payload = pending.get(key, (0, None))
                if st == 0:
                    pending[key] = (1, pc_s1(t0, ct))
                elif st == 1:
                    pending[key] = (2, pc_s2(t0, ct, payload))
                elif st == 2:
                    pending[key] = (3, pc_s3(t0, ct, payload, tail))
                elif st == 3:
                    pc_s4(t0, ct, *payload, tail=tail)
                    pending[key] = (4, None)

            sched = {}
            for q in range(3):
                for k in range(4):
                    sched.setdefault(4 * q + 5 + k, []).append((8 * q, 8))
            sched.setdefault(14, []).append((24, 4))
            sched.setdefault(15, []).append((24, 4))

            NU = NT // 2
            from collections import deque
            prevs = deque()
            for i in range(NU):
                prevs.append((i, phase_a1(i)))
                if i >= 1:
                    j, payload = prevs.popleft()
                    phase_a2(j, *payload)
                if i >= 4 and i % 4 == 0:
                    chain_q(8 * (i // 4 - 1))
                if i == 14:
                    chain_q(24, 4)
                for c in sched.get(i, []):
                    advance_chunk(c)
            while prevs:
                j, payload = prevs.popleft()
                phase_a2(j, *payload)
            chain_q(28, 4)
            for q in range(3):  # flush any unfinished in-loop chunks
                while pending.get((8 * q, 8), (0, None))[0] < 4:
                    advance_chunk((8 * q, 8))
            # tail: finish (24,4), run (28,4), 2-wide
            for _ in range(4):
                advance_chunk((28, 4), tail=True)
                if pending.get((24, 4), (0, None))[0] < 4:
                    advance_chunk((24, 4), tail=True)

    nc.compile()
    return nc


def _get_nc():
    global _BUILT
    if _BUILT is None:
        _BUILT = _build()
    return _BUILT


def kernel(a, b, add_table=None, carry_table=None, b2n=None, n2b=None, **_kw):
    from concourse.bass_utils import run_bass_kernel_spmd

    a = np.ascontiguousarray(np.asarray(a, dtype=np.float32))
    b = np.ascontiguousarray(np.asarray(b, dtype=np.float32))
    nc = _get_nc()
    in_maps = [
        {"a": a[i * B_CORE:(i + 1) * B_CORE], "b": b[i * B_CORE:(i + 1) * B_CORE]}
        for i in range(N_CORES)
    ]
    res = run_bass_kernel_spmd(nc, in_maps, core_ids=list(range(N_CORES)))
    out = np.concatenate([r["out"] for r in res.results], axis=0)
    return out.astype(np.float32)


# revision 36
# speedup vs baseline: 2.3641x; 1.0160x over previous
"""Trainium2 Bass kernel for nn_MoEALU (soft ripple-carry byte adder), v3.

Restructured math (validated in sim.py against the jax reference):
  - nibble sums: segmented sums of the 256-wide byte distribution per pos.
  - softmax1 kept UNNORMALIZED (te = exp(100(c - max))); the normalizer
    kappa_s = 1/(sum te_a * sum te_b) = 1/sum_m u_raw[m] is folded into the
    17 conv outputs afterwards.
  - cyclic conv u[m] = sum_i xa_i xb_{(m-i)%16} via a doubled-xb buffer
    (stride [+1 m, -1 i] AP); z15 == u[15].
  - Z1 = 1 - sum_i xa_i p[15-i] where p = prefix sums of xb (one scan).
  - carry chain: softmax over 2 == sigmoid => gamma' = sig(100(d + e*gamma)),
    d = Z1-Z0 = 1-2W, e = 2*z15. At temp 100 gamma saturates to {0,1}, so the
    recurrence linearizes EXACTLY (validated): gamma' = v0 + (v1-v0)*gamma
    with v0 = sig(100 d), v1 = sig(100(d+e)) => one tensor_tensor_scan with
    per-tile reset via zeroed b-coefficient.
  - s-logits: s = u + (rot1(u) - u)*gamma_in.
  - output softmax factorizes: softmax_256(100(sh_i + sl_j)) =
    softmax_16(100 sh) (x) softmax_16(100 sl): two 16-wide softmaxes and an
    outer product per byte; chain softmaxes use the fixed offset exp(100v-100).
fp16: te / conv products / u storage / A / final outer + DMA-out. exp outputs
eh/e2 stay fp32 (fp16 underflows for near-flat dists); per-side r2 folds
(r2h*r2l overflows fp32).

Schedule: phase A software-pipelined per tile; carry chain + phase C run per
16-tile half, with phase C emitted in 8-tile chunks x 4 stages interleaved
into the phase-A stream so DVE never stalls on ACT exps.

Sharding: pure data parallel over batch, 8 cores x 4096 rows.
"""

import numpy as np

B_FULL = 32768
N_CORES = 8
B_CORE = B_FULL // N_CORES  # 4096
P = 128
NT = B_CORE // P  # 32 tiles
HT = NT // 2  # tiles per half
CT = 8  # tiles per phase-C chunk

_BUILT = None


def _build():
    import concourse.bass as bass
    import concourse.bacc as bacc
    import concourse.mybir as mybir
    import concourse.tile as tile

    f32 = mybir.dt.float32
    f16 = mybir.dt.float16
    AF = mybir.ActivationFunctionType
    AX = mybir.AxisListType
    OP = mybir.AluOpType

    nc = bacc.Bacc("TRN2", target_bir_lowering=False, debug=False)
    a_d = nc.dram_tensor("a", [B_CORE, 4, 256], f32, kind="ExternalInput")
    b_d = nc.dram_tensor("b", [B_CORE, 4, 256], f32, kind="ExternalInput")
    out_d = nc.dram_tensor("out", [B_CORE, 4, 256], f16, kind="ExternalOutput")

    def ap(base_ap, off, dims):
        part = base_ap.ap[0]
        return bass.AP(base_ap.tensor, base_ap.offset + off,
                       [list(part)] + [list(d) for d in dims])

    with tile.TileContext(nc) as tc:
        with (
            tc.tile_pool(name="persist", bufs=1) as pp,
            tc.tile_pool(name="pin", bufs=2) as pin,
            tc.tile_pool(name="pa", bufs=2) as pa,
            tc.tile_pool(name="pa1", bufs=2) as pa1,
            tc.tile_pool(name="pc", bufs=2) as pc,
            tc.tile_pool(name="prep", bufs=2) as prep,
            tc.tile_pool(name="pout", bufs=3) as pout,
        ):
            # ---------------- persistent tensors ----------------
            u_all = pp.tile([P, NT, 8, 18], f16, tag="u_all")
            d_all = pp.tile([P, NT, 8], f32, tag="d_all")
            e_all = pp.tile([P, NT, 8], f32, tag="e_all")
            dpe = pp.tile([P, NT, 8], f32, tag="dpe")
            v0 = pp.tile([P, NT, 8], f32, tag="v0")
            v1 = pp.tile([P, NT, 8], f32, tag="v1")
            bco = pp.tile([P, NT, 8], f32, tag="bco")
            gg = pp.tile([P, 257], f32, tag="gg")
            g16 = pp.tile([P, NT, 8, 16], f16, tag="g16")
            nb100 = pp.tile([P, 1], f32, tag="nb100")
            nc.gpsimd.memset(nb100[:], -100.0)
            nc.gpsimd.memset(ap(bco[:], 0, [[8, NT]]), 0.0)

            a_v = a_d.ap().rearrange("(n p) f g -> n p (f g)", p=P)
            b_v = b_d.ap().rearrange("(n p) f g -> n p (f g)", p=P)
            o_v = out_d.ap().rearrange("(n p) f g -> n p (f g)", p=P)

            # ----- phase A stage 1 (2-tile super-tile): load+sums+max ----
            # ab layout [tensor, t, 1024] so (tensor,t) merges to stride 1024
            def phase_a1(u):
                ab = pin.tile([P, 2, 2, 1024], f32, tag="ab")
                order = ((0, 0), (0, 1), (1, 0), (1, 1)) if u == 0 else \
                    ((0, 0), (1, 0), (0, 1), (1, 1))
                for T2, t in order:
                    nc.sync.dma_start(
                        ap(ab[:], 2048 * T2 + 1024 * t, [[1, 1024]]),
                        (a_v if T2 == 0 else b_v)[2 * u + t])
                # c_all [tensor, t, 8 stages, 16]
                c_all = pa1.tile([P, 512], f32, tag="c_all")
                lo1 = pa.tile([P, 4, 4, 16, 8], f32, tag="lo1")
                if u == 0:
                    # split per tensor-half so compute starts after 2 DMAs
                    for T2 in range(2):
                        nc.vector.tensor_reduce(
                            ap(c_all[:], 256 * T2 + 16,
                               [[128, 2], [32, 4], [1, 16]]),
                            ap(ab[:], 2048 * T2,
                               [[1024, 2], [256, 4], [16, 16], [1, 16]]),
                            axis=AX.X, op=OP.add)
                        nc.gpsimd.tensor_add(
                            ap(lo1[:], 1024 * T2,
                               [[512, 2], [128, 4], [8, 16], [1, 8]]),
                            ap(ab[:], 2048 * T2,
                               [[1024, 2], [256, 4], [1, 16], [16, 8]]),
                            ap(ab[:], 2048 * T2 + 128,
                               [[1024, 2], [256, 4], [1, 16], [16, 8]]))
                else:
                    nc.vector.tensor_reduce(
                        ap(c_all[:], 16, [[128, 4], [32, 4], [1, 16]]),
                        ap(ab[:], 0, [[1024, 4], [256, 4], [16, 16], [1, 16]]),
                        axis=AX.X, op=OP.add)
                    nc.gpsimd.tensor_add(
                        lo1[:],
                        ap(ab[:], 0, [[1024, 4], [256, 4], [1, 16], [16, 8]]),
                        ap(ab[:], 128, [[1024, 4], [256, 4], [1, 16], [16, 8]]))
                lo2 = pa.tile([P, 4, 4, 16, 4], f32, tag="lo2")
                nc.gpsimd.tensor_add(
                    lo2[:],
                    ap(lo1[:], 0, [[512, 4], [128, 4], [8, 16], [1, 4]]),
                    ap(lo1[:], 4, [[512, 4], [128, 4], [8, 16], [1, 4]]))
                lo3 = pa.tile([P, 4, 4, 16, 2], f32, tag="lo3")
                nc.gpsimd.tensor_add(
                    lo3[:],
                    ap(lo2[:], 0, [[256, 4], [64, 4], [4, 16], [1, 2]]),
                    ap(lo2[:], 2, [[256, 4], [64, 4], [4, 16], [1, 2]]))
                nc.gpsimd.tensor_add(
                    ap(c_all[:], 0, [[128, 4], [32, 4], [1, 16]]),
                    ap(lo3[:], 0, [[128, 4], [32, 4], [2, 16]]),
                    ap(lo3[:], 1, [[128, 4], [32, 4], [2, 16]]))
                m16 = pa1.tile([P, 32], f32, tag="m16")
                nc.vector.tensor_reduce(
                    m16[:], c_all[:].rearrange("p (g e) -> p g e", g=32),
                    axis=AX.X, op=OP.max)
                return c_all, m16

            # ----- phase A stage 2 (2-tile super-tile) -----
            def phase_a2(u, c_all, m16):
                ts = pa.tile([P, 512], f32, tag="ts")
                nc.gpsimd.tensor_sub(
                    ts[:].rearrange("p (g e) -> p g e", g=32),
                    c_all[:].rearrange("p (g e) -> p g e", g=32),
                    ap(m16[:], 0, [[1, 32], [0, 16]]))
                # T [tensor, t, s, 16]: b-half contiguous at 256 for the scan
                T = pa.tile([P, 2, 2, 8, 16], f16, tag="T")
                nc.scalar.activation(
                    T[:].rearrange("p a b c d -> p (a b c d)"),
                    ts[:], AF.Exp, scale=100.0)
                xbd = pa.tile([P, 2, 8, 32], f16, tag="xbd")
                nc.scalar.copy(
                    ap(xbd[:], 0, [[256, 2], [32, 8], [1, 16]]),
                    ap(T[:], 256, [[128, 2], [16, 8], [1, 16]]))
                nc.scalar.copy(
                    ap(xbd[:], 16, [[256, 2], [32, 8], [1, 16]]),
                    ap(T[:], 256, [[128, 2], [16, 8], [1, 16]]))
                S = pa.tile([P, 260], f32, tag="S")
                nc.gpsimd.memset(ap(S[:], 0, [[1, 1]]), 0.0)
                nc.vector.tensor_tensor_scan(
                    ap(S[:], 1, [[1, 256]]),
                    ap(T[:], 256, [[1, 256]]),
                    ap(T[:], 256, [[1, 256]]),
                    0.0, OP.add, OP.bypass)
                p16 = pa.tile([P, 2, 8, 16], f16, tag="p16")
                nc.vector.tensor_sub(
                    p16[:],
                    ap(S[:], 1, [[16, 16], [1, 16]]),
                    ap(S[:], 0, [[16, 16], [0, 16]]))
                q = pa.tile([P, 2, 8, 17, 16], f16, tag="q")
                nc.vector.tensor_mul(
                    ap(q[:], 0, [[2176, 2], [272, 8], [16, 16], [1, 16]]),
                    ap(T[:], 0, [[128, 2], [16, 8], [0, 16], [1, 16]]),
                    ap(xbd[:], 16, [[256, 2], [32, 8], [1, 16], [-1, 16]]))
                nc.vector.tensor_mul(
                    ap(q[:], 256, [[2176, 2], [272, 8], [1, 16]]),
                    ap(T[:], 0, [[128, 2], [16, 8], [1, 16]]),
                    ap(p16[:], 15, [[128, 2], [16, 8], [-1, 16]]))
                qt1 = pa.tile([P, 2, 8, 17, 8], f16, tag="qt1")
                nc.vector.tensor_add(
                    qt1[:],
                    ap(q[:], 0, [[2176, 2], [272, 8], [16, 17], [1, 8]]),
                    ap(q[:], 8, [[2176, 2], [272, 8], [16, 17], [1, 8]]))
                qt2 = pa.tile([P, 2, 8, 17, 4], f16, tag="qt2")
                nc.vector.tensor_add(
                    qt2[:],
                    ap(qt1[:], 0, [[1088, 2], [136, 8], [8, 17], [1, 4]]),
                    ap(qt1[:], 4, [[1088, 2], [136, 8], [8, 17], [1, 4]]))
                qt3 = pa.tile([P, 2, 8, 17, 2], f16, tag="qt3")
                nc.vector.tensor_add(
                    qt3[:],
                    ap(qt2[:], 0, [[544, 2], [68, 8], [4, 17], [1, 2]]),
                    ap(qt2[:], 2, [[544, 2], [68, 8], [4, 17], [1, 2]]))
                qr = pa.tile([P, 2, 8, 17], f16, tag="qr")
                nc.vector.tensor_add(
                    qr[:],
                    ap(qt3[:], 0, [[272, 2], [34, 8], [2, 17]]),
                    ap(qt3[:], 1, [[272, 2], [34, 8], [2, 17]]))
                Su = pa.tile([P, 16], f32, tag="Su")
                nc.vector.tensor_reduce(
                    Su[:], ap(qr[:], 0, [[17, 16], [1, 16]]),
                    axis=AX.X, op=OP.add)
                rk = pa.tile([P, 16], f32, tag="rk")
                nc.vector.reciprocal(rk[:], Su[:])
                nc.gpsimd.tensor_mul(
                    ap(u_all[:], 288 * u + 1, [[144, 2], [18, 8], [1, 16]]),
                    ap(qr[:], 0, [[136, 2], [17, 8], [1, 16]]),
                    ap(rk[:], 0, [[8, 2], [1, 8], [0, 16]]))
                nc.gpsimd.tensor_mul(
                    ap(u_all[:], 288 * u, [[144, 2], [18, 8]]),
                    ap(qr[:], 15, [[136, 2], [17, 8]]),
                    ap(rk[:], 0, [[8, 2], [1, 8]]))
                t8 = pa.tile([P, 16], f32, tag="t8")
                nc.gpsimd.tensor_mul(
                    t8[:], ap(qr[:], 16, [[136, 2], [17, 8]]),
                    ap(rk[:], 0, [[8, 2], [1, 8]]))
                nc.vector.tensor_scalar(
                    ap(d_all[:], 16 * u, [[1, 16]]), t8[:],
                    -2.0, 1.0, op0=OP.mult, op1=OP.add)
                nc.vector.tensor_scalar_mul(
                    ap(e_all[:], 16 * u, [[1, 16]]),
                    ap(u_all[:], 288 * u, [[144, 2], [18, 8]]), 2.0)

            # --------- carry chain for tiles [t0, t0+nt) ---------
            def chain_q(t0, nt=8):
                o = 8 * t0
                dsl = ap(d_all[:], o, [[1, 8 * nt]])
                esl = ap(e_all[:], o, [[1, 8 * nt]])
                psl = ap(dpe[:], o, [[1, 8 * nt]])
                v0s = ap(v0[:], o, [[1, 8 * nt]])
                v1s = ap(v1[:], o, [[1, 8 * nt]])
                nc.gpsimd.tensor_add(psl, dsl, esl)
                nc.scalar.activation(v0s, dsl, AF.Sigmoid, scale=100.0)
                nc.scalar.activation(v1s, psl, AF.Sigmoid, scale=100.0)
                nc.gpsimd.tensor_sub(
                    ap(bco[:], o + 1, [[8, nt], [1, 7]]),
                    ap(v1[:], o + 1, [[8, nt], [1, 7]]),
                    ap(v0[:], o + 1, [[8, nt], [1, 7]]))
                nc.vector.tensor_tensor_scan(
                    ap(gg[:], o + 1, [[1, 8 * nt]]),
                    ap(bco[:], o, [[1, 8 * nt]]),
                    ap(v0[:], o, [[1, 8 * nt]]),
                    0.0, OP.mult, OP.add)
                # gg[o+8t] slots hold only discarded stage-7 carry-outs
                nc.gpsimd.memset(ap(gg[:], o, [[8, nt]]), 0.0)
                nc.scalar.copy(
                    ap(g16[:], 128 * t0, [[128, nt], [16, 8], [1, 16]]),
                    ap(gg[:], o, [[8, nt], [1, 8], [0, 16]]))

            # --------- phase C chunk stages (CT=8 tiles each) ---------
            def pc_s1(t0, ct):
                o = 144 * t0
                u_sl = ap(u_all[:], o + 1, [[144, ct], [18, 8], [1, 16]])
                rot_sl = ap(u_all[:], o, [[144, ct], [18, 8], [1, 16]])
                dlt = pc.tile([P, ct, 8, 16], f16, tag="dlt")
                tb = pc.tile([P, ct, 8, 16], f16, tag="tb")
                sb = pc.tile([P, ct, 8, 16], f16, tag="sb")
                eh = pc.tile([P, ct, 8, 16], f32, tag="eh")
                nc.vector.tensor_sub(dlt[:], rot_sl, u_sl)
                nc.vector.tensor_mul(
                    tb[:], dlt[:],
                    ap(g16[:], 128 * t0, [[128, ct], [16, 8], [1, 16]]))
                nc.vector.tensor_add(sb[:], u_sl, tb[:])
                nc.scalar.activation(eh[:], sb[:], AF.Exp,
                                     bias=nb100[:], scale=100.0)
                return eh

            def pc_s2(t0, ct, eh):
                ns = pc.tile([P, ct, 8], f32, tag="ns")
                r1 = pc.tile([P, ct, 8], f32, tag="r1")
                A16 = pc.tile([P, ct, 8, 16], f16, tag="A16")
                e2 = pc.tile([P, ct, 8, 16], f32, tag="e2")
                nc.vector.tensor_reduce(
                    ns[:], eh[:].rearrange("p a b e -> p (a b) e"),
                    axis=AX.X, op=OP.add)
                nc.vector.reciprocal(r1[:], ns[:])
                nc.gpsimd.tensor_mul(
                    A16[:], eh[:],
                    ap(r1[:], 0, [[8, ct], [1, 8], [0, 16]]))
                nc.scalar.activation(e2[:], A16[:], AF.Exp,
                                     bias=nb100[:], scale=100.0)
                return e2

            def pc_s3(t0, ct, e2, tail=False):
                s2 = pc.tile([P, ct, 8], f32, tag="s2")
                r2 = pc.tile([P, ct, 8], f32, tag="r2")
                e2h = pc.tile([P, ct, 4, 16], f16, tag="e2h")
                e2l = pc.tile([P, ct, 4, 16], f16, tag="e2l")
                nc.vector.tensor_reduce(
                    s2[:], e2[:].rearrange("p a b e -> p (a b) e"),
                    axis=AX.X, op=OP.add)
                nc.vector.reciprocal(r2[:], s2[:])
                nc.vector.tensor_mul(
                    e2h[:],
                    ap(e2[:], 16, [[128, ct], [32, 4], [1, 16]]),
                    ap(r2[:], 1, [[8, ct], [2, 4], [0, 16]]))
                eng_l = nc.vector if tail else nc.gpsimd
                eng_l.tensor_mul(
                    e2l[:],
                    ap(e2[:], 0, [[128, ct], [32, 4], [1, 16]]),
                    ap(r2[:], 0, [[8, ct], [2, 4], [0, 16]]))
                return e2h, e2l

            def pc_s4(t0, ct, e2h, e2l, tail=False):
                for gsub in range(ct // 4):
                    rep = prep.tile([P, 4, 4, 16, 16], f16, tag="rep")
                    nc.scalar.copy(
                        rep[:],
                        ap(e2h[:], 256 * gsub,
                           [[64, 4], [16, 4], [1, 16], [0, 16]]))
                    for t in range(4):
                        i = t0 + 4 * gsub + t
                        o_t = pout.tile([P, 4, 16, 16], f16, tag="o_t")
                        nc.vector.tensor_mul(
                            o_t[:],
                            ap(rep[:], 1024 * t, [[256, 4], [16, 16], [1, 16]]),
                            ap(e2l[:], 64 * (4 * gsub + t),
                               [[16, 4], [0, 16], [1, 16]]))
                        nc.scalar.dma_start(
                            o_v[i], o_t[:].rearrange("p a b c -> p (a b c)"))

            # ================= emission schedule =================
            # software-pipelined phase A; carry chain per 8-tile quarter;
            # phase-C chunk q stages spread over tiles 8q+9 .. 8q+15 so only
            # the last quarter's chunk runs after phase A ends.
            pending = {}  # c -> (stage, payload)

            def advance_chunk(key, tail=False):
                t0, ct = key
                st, payload = pending.get(key, (0, None))
                if st == 0:
                    pending[key] = (1, pc_s1(t0, ct))
                elif st == 1:
                    pending[key] = (2, pc_s2(t0, ct, payload))
                elif st == 2:
                    pending[key] = (3, pc_s3(t0, ct, payload, tail))
                elif st == 3:
                    pc_s4(t0, ct, *payload, tail=tail)
                    pending[key] = (4, None)

            sched = {}
            for q in range(3):
                for k in range(4):
                    sched.setdefault(4 * q + 5 + k, []).append((8 * q, 8))
            sched.setdefault(14, []).append((24, 4))
            sched.setdefault(15, []).append((24, 4))

            NU = NT // 2
            from collections import deque
            prevs = deque()
            for i in range(NU):
                if i >= 1 and prevs:
                    j, payload = prevs.popleft()
                    phase_a2(j, *payload)
                if i >= 4 and i % 4 == 0:
                    chain_q(8 * (i // 4 - 1))
                if i == 14:
                    chain_q(24, 4)
                for c in sched.get(i, []):
                    advance_chunk(c)
                prevs.append((i, phase_a1(i)))
            while prevs:
                j, payload = prevs.popleft()
                phase_a2(j, *payload)
            chain_q(28, 4)
            for q in range(3):  # flush any unfinished in-loop chunks
                while pending.get((8 * q, 8), (0, None))[0] < 4:
                    advance_chunk((8 * q, 8))
            # tail: finish (24,4), run (28,4), 2-wide
            for _ in range(4):
                advance_chunk((28, 4), tail=True)
                if pending.get((24, 4), (0, None))[0] < 4:
                    advance_chunk((24, 4), tail=True)

    nc.compile()
    return nc


def _get_nc():
    global _BUILT
    if _BUILT is None:
        _BUILT = _build()
    return _BUILT


def kernel(a, b, add_table=None, carry_table=None, b2n=None, n2b=None, **_kw):
    from concourse.bass_utils import run_bass_kernel_spmd

    a = np.ascontiguousarray(np.asarray(a, dtype=np.float32))
    b = np.ascontiguousarray(np.asarray(b, dtype=np.float32))
    nc = _get_nc()
    in_maps = [
        {"a": a[i * B_CORE:(i + 1) * B_CORE], "b": b[i * B_CORE:(i + 1) * B_CORE]}
        for i in range(N_CORES)
    ]
    res = run_bass_kernel_spmd(nc, in_maps, core_ids=list(range(N_CORES)))
    out = np.concatenate([r["out"] for r in res.results], axis=0)
    return out.astype(np.float32)
